# revision 1
# baseline (speedup 1.0000x reference)
"""Trainium2 Bass kernel for nn_Attention_11579231830437.

Masked multi-head attention (ReLU'd QKV projections, additive key mask,
multiplicative query mask) followed by training-mode BatchNorm over (B, T).

Strategy: data-parallel over batch B across 8 NeuronCores (4 batches each).
The host compacts each batch's sequence to its valid (mask==1) positions —
both attention masks zero out the same positions, so attention is computed
only on the ~50% valid positions (sorted batch->slot assignment keeps the
per-slot padded length tight). BatchNorm mean/var sums are all-reduced
across cores on-chip (single collective at the end, with a warm-up
collective at kernel start to absorb ncfw first-trigger latency);
normalization is applied on-device before gathering.

On-device layout per batch slot (Tj = padded valid length of slot j):
  QT, KT   [C, Tj]   channel-major (per-partition bias + relu on ACT)
  V        [t, (h, dv | ones)]  row-major with a ones column per head, so the
           attention@V matmul also emits the softmax denominator row D.
  scores   ST[s, t] = KhT.T @ QhT per head (row-group tiled, K=32)
  P        exp(scale*ST + key_bias[s])  (key mask folded into the exp bias)
  out      O'[dv, t] accumulated over s-chunks; x = O' * (qmask/D) broadcast
           via a block-ones matmul; BN stats accumulate via fused reduces.
"""

import os
import sys
import types
from contextlib import ExitStack

import numpy as np

# Defensive: concourse.bass_utils imports antenv.axon_hooks when tracing is
# requested via env; provide a no-op holder if the image lacks it.
try:
    import antenv.axon_hooks  # noqa: F401
except Exception:
    try:
        import antenv

        _m = types.ModuleType("antenv.axon_hooks")
        _m._hook = None
        _m.set_axon_ntff_profile_hook = lambda h: setattr(_m, "_hook", h)
        _m.get_axon_ntff_profile_hook = lambda: getattr(_m, "_hook", None)
        sys.modules["antenv.axon_hooks"] = _m
        antenv.axon_hooks = _m
    except Exception:
        pass

import concourse.bass as bass
import concourse.bacc as bacc
import concourse.tile as tile
from concourse import mybir
from concourse.bass_utils import run_bass_kernel_spmd

f32 = mybir.dt.float32
bf16 = mybir.dt.bfloat16
FT = mybir.ActivationFunctionType
ALU = mybir.AluOpType

N_CORES = 8
B, C, T, H = 32, 256, 512, 8
D = C // H                      # 32 per-head dim
NB = B // N_CORES               # 4 batch slots per core
G = C // 128                    # 2 channel chunks of 128
EPS = 1e-5
SCALE = 1.0 / float(np.sqrt(D))
KB_NEG = -200.0                 # exp(-200 + few) == 0.0 in fp32
INV_BT = 1.0 / float(B * T)


def _cdiv(a, b):
    return (a + b - 1) // b


def _build(slot_T):
    """Build the SPMD program for per-slot padded lengths slot_T (NB ints)."""
    USE_CC = os.environ.get("K_NO_CC", "") == ""
    Tmax = max(slot_T)
    # slots whose 3rd key-chunk is tiny (<=32 keys): those keys are packed
    # into 32-row blocks (one per head quadrant) so the chunk costs one
    # exp of free-size Tj instead of four of 2*Tj.
    TAIL = {j for j in range(NB)
            if _cdiv(slot_T[j], 128) == 3 and slot_T[j] - 256 <= 32}
    nc = bacc.Bacc("TRN2", target_bir_lowering=False, debug=False,
                   num_devices=N_CORES)

    seq_d, kb_d, qm_d, out_d = [], [], [], []
    kbt_d = {}
    for j in range(NB):
        Tj = slot_T[j]
        sch = _cdiv(Tj, 128)
        seq_d.append(nc.dram_tensor(f"seq{j}", [C, Tj], bf16,
                                    kind="ExternalInput").ap())
        kb_d.append(nc.dram_tensor(f"kb{j}", [sch * 128], f32,
                                   kind="ExternalInput").ap())
        qm_d.append(nc.dram_tensor(f"qm{j}", [1, Tj], f32,
                                   kind="ExternalInput").ap())
        out_d.append(nc.dram_tensor(f"out{j}", [C, Tj], f32,
                                    kind="ExternalOutput").ap())
        if j in TAIL:
            kbt_d[j] = nc.dram_tensor(f"kbt{j}", [128], f32,
                                      kind="ExternalInput").ap()

    wall_d = nc.dram_tensor("wall", [C, 3 * C], bf16, kind="ExternalInput").ap()
    ball_d = nc.dram_tensor("ball", [4 * C], f32, kind="ExternalInput").ap()
    bvr_d = nc.dram_tensor("bvr", [1, C], bf16, kind="ExternalInput").ap()
    bo_d = nc.dram_tensor("bones", [128, 128], bf16, kind="ExternalInput").ap()
    or_d = nc.dram_tensor("onesr", [1, 128], bf16, kind="ExternalInput").ap()
    on_d = nc.dram_tensor("ones2", [1, 2 * H], f32, kind="ExternalInput").ap()

    with tile.TileContext(nc) as tc, ExitStack() as ctx:
        const = ctx.enter_context(tc.tile_pool(name="const", bufs=1))
        seqp = ctx.enter_context(tc.tile_pool(name="seqp", bufs=3))
        qkp = ctx.enter_context(tc.tile_pool(name="qkp", bufs=3))
        vp = ctx.enter_context(tc.tile_pool(name="vp", bufs=3))
        pp = ctx.enter_context(tc.tile_pool(name="pp", bufs=3))
        xp = ctx.enter_context(tc.tile_pool(name="xp", bufs=NB))
        smallp = ctx.enter_context(tc.tile_pool(name="smallp", bufs=2))
        fbp = ctx.enter_context(tc.tile_pool(name="fbp", bufs=2))
        statp = ctx.enter_context(tc.tile_pool(name="statp", bufs=1))
        outp = ctx.enter_context(tc.tile_pool(name="outp", bufs=4))
        scrp = ctx.enter_context(tc.tile_pool(name="scrp", bufs=2))
        ps_proj = ctx.enter_context(tc.tile_pool(name="ps_proj", bufs=1, space="PSUM"))
        ps_sc = ctx.enter_context(tc.tile_pool(name="ps_sc", bufs=2, space="PSUM"))
        ps_av = ctx.enter_context(tc.tile_pool(name="ps_av", bufs=2, space="PSUM"))
        ps_fb = ctx.enter_context(tc.tile_pool(name="ps_fb", bufs=1, space="PSUM"))
        dramp = ctx.enter_context(tc.tile_pool(name="dramp", bufs=1, space="DRAM"))

        # ---- constants ----
        wpack = [const.tile([128, 3 * C], bf16, tag=f"wp{k}", name=f"wp{k}") for k in range(G)]
        nc.sync.dma_start(wpack[0][:], wall_d[0:128, :])
        nc.scalar.dma_start(wpack[1][:], wall_d[128:256, :])
        bpack = const.tile([128, 4, G], f32, tag="bpack")
        nc.sync.dma_start(
            bpack[:],
            bass.AP(tensor=ball_d.tensor, offset=ball_d.offset,
                    ap=[[1, 128], [128 * G, 4], [128, G]]),
        )
        bq_t = bpack[:, 0, :]
        bk_t = bpack[:, 1, :]
        gm_t = bpack[:, 2, :]
        bt_t = bpack[:, 3, :]
        wq_s = [wpack[k][:, 0:C] for k in range(G)]
        wk_s = [wpack[k][:, C:2 * C] for k in range(G)]
        wv_s = [wpack[k][:, 2 * C:3 * C] for k in range(G)]
        bones_b = const.tile([128, 128], bf16, tag="bones")
        nc.scalar.dma_start(bones_b[:], bo_d[:])
        bv_r = const.tile([1, C], bf16, tag="bvr32")
        nc.scalar.dma_start(bv_r[:], bvr_d[:])
        onesr = const.tile([1, 128], bf16, tag="onesr")
        nc.scalar.dma_start(onesr[:], or_d[:])
        eps_t = const.tile([128, 1], f32, tag="eps")
        nc.vector.memset(eps_t[:], EPS)
        zero_t = const.tile([128, Tmax], f32, tag="zerot")
        nc.gpsimd.memset(zero_t[:], 0.0)
        ones2_b = const.tile([128, H, 2], f32, tag="ones2b")
        nc.sync.dma_start(
            ones2_b[:],
            bass.AP(tensor=on_d.tensor, offset=on_d.offset,
                    ap=[[0, 128], [2, H], [1, 2]]),
        )

        st1 = [statp.tile([128, NB], f32, tag=f"st1_{g}", name=f"st1_{g}") for g in range(G)]
        st2 = [statp.tile([128, NB], f32, tag=f"st2_{g}", name=f"st2_{g}") for g in range(G)]

        # Warm-up collective: absorbs ncfw collectives-firmware warm-up off
        # the critical path (result unused).
        if USE_CC:
            warm_sb = const.tile([128, 2], f32, tag="warmsb")
            nc.vector.memset(warm_sb[:], 0.0)
            warm_in = dramp.tile([128, 2], f32, tag="warmin")
            warm_out = dramp.tile([128, 2], f32, tag="warmout")
            nc.sync.dma_start(warm_in[:], warm_sb[:])
            nc.gpsimd.collective_compute(
                "AllReduce", ALU.add,
                replica_groups=[list(range(N_CORES))],
                ins=[warm_in[:]], outs=[warm_out[:]],
            )

        xts = []  # [sl][g] -> XT tile
        S = [dict() for _ in range(NB)]

        def phase1(sl):
            Tj = slot_T[sl]
            sch = _cdiv(Tj, 128)
            msz = [min(128, Tj - 128 * i) for i in range(sch)]
            st = S[sl]
            st["sch"], st["msz"] = sch, msz

            s_in = [seqp.tile([128, Tmax], bf16, tag=f"sin{k}", name=f"sin{k}_{sl}") for k in range(G)]
            for k, eng in ((0, nc.sync), (1, nc.scalar)):
                eng.dma_start(s_in[k][:, :Tj],
                              seq_d[sl][128 * k:128 * (k + 1), :])
            kb_t = smallp.tile([128, 4], f32, tag="kb", name=f"kb_{sl}")
            nc.scalar.dma_start(
                kb_t[:, :sch],
                bass.AP(tensor=kb_d[sl].tensor, offset=kb_d[sl].offset,
                        ap=[[1, 128], [128, sch]]),
            )
            qm_b = smallp.tile([128, Tmax], f32, tag="qmb", name=f"qmb_{sl}")
            nc.sync.dma_start(
                qm_b[:, :Tj],
                bass.AP(tensor=qm_d[sl].tensor, offset=qm_d[sl].offset,
                        ap=[[0, 128], [1, Tj]]),
            )
            st["kb_t"], st["qm_b"] = kb_t, qm_b

            qt = [qkp.tile([128, Tmax], bf16, tag=f"qt{g}", name=f"qt{g}_{sl}") for g in range(G)]
            kt = [qkp.tile([128, Tmax], bf16, tag=f"kt{g}", name=f"kt{g}_{sl}") for g in range(G)]
            for g in range(G):
                for (w_s, b_t, dst) in ((wq_s, bq_t, qt), (wk_s, bk_t, kt)):
                    psq = ps_proj.tile([128, 512], f32, tag="psproj",
                                       name=f"psq_{sl}_{g}")
                    for k in range(G):
                        nc.tensor.matmul(
                            psq[:, :Tj],
                            w_s[k][:, 128 * g:128 * (g + 1)],
                            s_in[k][:, :Tj],
                            start=(k == 0), stop=(k == G - 1),
                        )
                    # relu(psq + bias) on DVE: (in0 + bias) max 0 — keeps the
                    # busier ACT engine free for exp.
                    nc.vector.scalar_tensor_tensor(
                        out=dst[g][:, :Tj], in0=psq[:, :Tj],
                        scalar=b_t[:, g:g + 1], in1=zero_t[:, :Tj],
                        op0=ALU.add, op1=ALU.max,
                    )
            st["qt"], st["kt"] = qt, kt

            v_t = [vp.tile([128, H, D + 2], bf16, tag=f"vt{i}", name=f"vt{i}_{sl}") for i in range(sch)]
            for i in range(sch):
                m = msz[i]
                psv = ps_proj.tile([128, 512], f32, tag="psproj",
                                   name=f"psv_{sl}_{i}")
                for k in range(G):
                    nc.tensor.matmul(
                        psv[:m, :C],
                        s_in[k][:, 128 * i:128 * i + m],
                        wv_s[k],
                        start=(k == 0), stop=False,
                    )
                nc.tensor.matmul(
                    psv[:m, :C], onesr[:, :m], bv_r[:],
                    start=False, stop=True,
                )
                nc.vector.tensor_scalar_max(
                    out=v_t[i][:m, :, 0:D],
                    in0=psv[:m, :C].rearrange("p (h d) -> p h d", h=H),
                    scalar1=0.0,
                )
                nc.vector.tensor_copy(v_t[i][:m, :, D:D + 2], ones2_b[:m, :, :])
            st["v_t"] = v_t
            if sl in TAIL:
                kbt_t = smallp.tile([128, 1], f32, tag="kbt", name=f"kbt_{sl}")
                nc.scalar.dma_start(
                    kbt_t[:],
                    bass.AP(tensor=kbt_d[sl].tensor, offset=kbt_d[sl].offset,
                            ap=[[1, 128], [128, 1]]),
                )
                # replicate the tail-chunk V rows to all four 32-row blocks
                vrep = vp.tile([128, H, D + 2], bf16, tag="vrep",
                               name=f"vrep_{sl}")
                vt = v_t[sch - 1]
                m_t = msz[sch - 1]
                for a in range(4):
                    (nc.sync if a % 2 == 0 else nc.scalar).dma_start(
                        vrep[32 * a:32 * a + m_t, :, :], vt[:m_t, :, :])
                st["kbt_t"], st["vrep"] = kbt_t, vrep

        def phase2(sl):
            Tj = slot_T[sl]
            st = S[sl]
            sch, msz = st["sch"], st["msz"]
            qt, kt, kb_t = st["qt"], st["kt"], st["kb_t"]
            has_tail = sl in TAIL
            ii = sch - 1 if has_tail else sch
            P = {}
            for i in range(ii):
                m = msz[i]
                for g in range(G):
                    for p_ in range(2):
                        ps2 = ps_sc.tile([128, 2, 512], f32, tag="pssc",
                                         name=f"ps2_{sl}_{i}_{g}_{p_}")
                        for jj in range(2):
                            j = 2 * p_ + jj
                            nc.tensor.matmul(
                                ps2[:m, jj, :Tj],
                                kt[g][32 * j:32 * (j + 1), 128 * i:128 * i + m],
                                qt[g][32 * j:32 * (j + 1), :Tj],
                                start=True, stop=True,
                                tile_position=(32 * j, 0),
                            )
                        pt = pp.tile([128, 2, Tmax], bf16, tag=f"p{i}{g}{p_}",
                                     name=f"p{i}{g}{p_}_{sl}")
                        nc.scalar.activation(
                            pt[:m, :, :Tj], ps2[:m, :, :Tj], FT.Exp,
                            bias=kb_t[:m, i:i + 1], scale=SCALE,
                        )
                        P[(i, g, 2 * p_)] = (pt, 0)
                        P[(i, g, 2 * p_ + 1)] = (pt, 1)
            st["P"] = P
            if has_tail:
                # packed tail round: head (g,j)'s <=32 tail keys land on
                # partitions 32j..; one exp of free-size Tj covers all four.
                # Unwritten partitions get bias -200 -> P=0.
                m_t = msz[sch - 1]
                kbt_t = st["kbt_t"]
                Pt = []
                for g in range(G):
                    pst = ps_sc.tile([128, 2, 512], f32, tag="pssc",
                                     name=f"pst_{sl}_{g}")
                    for j in range(4):
                        nc.tensor.matmul(
                            pst[32 * j:32 * j + m_t, 0, :Tj],
                            kt[g][32 * j:32 * (j + 1), 256:256 + m_t],
                            qt[g][32 * j:32 * (j + 1), :Tj],
                            start=True, stop=True,
                            tile_position=(32 * j, 32 * j),
                        )
                    ptt = pp.tile([128, Tmax], bf16, tag=f"ptt{g}",
                                  name=f"ptt{g}_{sl}")
                    nc.scalar.activation(
                        ptt[:, :Tj], pst[:, 0, :Tj], FT.Exp,
                        bias=kbt_t[:, 0:1], scale=SCALE,
                    )
                    Pt.append(ptt)
                st["Pt"] = Pt

        def phase3(sl):
            Tj = slot_T[sl]
            st = S[sl]
            sch, msz = st["sch"], st["msz"]
            v_t, P, qm_b = st["v_t"], st["P"], st["qm_b"]
            has_tail = sl in TAIL
            ii = sch - 1 if has_tail else sch
            m_t = msz[sch - 1] if has_tail else 0
            xt = [xp.tile([128, Tmax], f32, tag=f"xt{g}", name=f"xt{g}_{sl}") for g in range(G)]
            xts.append(xt)
            d_all = smallp.tile([128, Tmax], f32, tag="dall", name=f"dall_{sl}")
            nc.gpsimd.memset(d_all[:], 1.0)
            r_all = smallp.tile([128, Tmax], f32, tag="rall", name=f"rall_{sl}")
            f_all = smallp.tile([128, Tmax], bf16, tag="fall", name=f"fall_{sl}")
            a_sbs = {}
            for g in range(G):
                for p in range(2):
                    r = 2 * g + p
                    psA = ps_av.tile([128, 512], f32, tag="psav", name=f"psav{g}{p}_{sl}")
                    for pp_ in range(2):
                        h = 4 * g + 2 * p + pp_
                        base = 64 * pp_
                        j = 2 * p + pp_
                        for i in range(ii):
                            m = msz[i]
                            pt, jj = P[(i, g, 2 * p + pp_)]
                            nc.tensor.matmul(
                                psA[base:base + D + 2, :Tj],
                                v_t[i][:m, h, :],
                                pt[:m, jj, :Tj],
                                start=(i == 0),
                                stop=(i == ii - 1 and not has_tail),
                                tile_position=(0, base),
                            )
                        if has_tail:
                            nc.tensor.matmul(
                                psA[base:base + D + 2, :Tj],
                                st["vrep"][32 * j:32 * j + m_t, h, :],
                                st["Pt"][g][32 * j:32 * j + m_t, :Tj],
                                start=False, stop=True,
                                tile_position=(32 * j, base),
                            )
                    asb = fbp.tile([128, Tmax], f32, tag=f"asb{r}",
                                   name=f"asb{r}_{sl}")
                    a_sbs[r] = asb
                    if r % 2 == 0:
                        nc.vector.tensor_copy(asb[:, :Tj], psA[:, :Tj])
                    else:
                        nc.scalar.activation(asb[:, :Tj], psA[:, :Tj],
                                             FT.Copy)
                    pitch = asb.ap[0][0]
                    dsrc = bass.AP(
                        tensor=asb.tensor,
                        offset=asb.offset + D * pitch,
                        ap=[[64 * pitch, 2], [1, Tj]],
                    )
                    nc.sync.dma_start(d_all[32 * r:32 * r + 2, :Tj], dsrc)
            # D > 0 always holds: every query column (valid or padded) sees at
            # least one valid key with P >= exp(-|s|) > 0, so no eps guard.
            nc.vector.reciprocal(r_all[:, :Tj], d_all[:, :Tj])
            nc.vector.tensor_tensor(out=f_all[:, :Tj], in0=r_all[:, :Tj],
                                    in1=qm_b[:, :Tj], op=ALU.mult)
            for g in range(G):
                for p in range(2):
                    r = 2 * g + p
                    asb = a_sbs[r]
                    psF = ps_fb.tile([128, 512], f32, tag="psfb",
                                     name=f"psF_{r}_{sl}")
                    nc.tensor.matmul(psF[:, :Tj],
                                     bones_b[32 * r:32 * r + 2, :],
                                     f_all[32 * r:32 * r + 2, :Tj],
                                     start=True, stop=True,
                                     tile_position=(32 * r, 0))
                    for pp_ in range(2):
                        j = 2 * p + pp_
                        nc.vector.scalar_tensor_tensor(
                            out=xt[g][32 * j:32 * (j + 1), :Tj],
                            in0=asb[64 * pp_:64 * pp_ + D, :Tj],
                            scalar=1.0,
                            in1=psF[64 * pp_:64 * pp_ + D, :Tj],
                            op0=ALU.mult, op1=ALU.mult,
                            accum_out=st1[g][32 * j:32 * (j + 1), sl:sl + 1],
                        )
                scr = scrp.tile([128, Tmax], f32, tag="scr",
                                name=f"scr_{sl}_{g}")
                nc.scalar.activation(
                    scr[:, :Tj], xt[g][:, :Tj], FT.Square,
                    accum_out=st2[g][:, sl:sl + 1],
                )

        # software-pipelined emission: next slot's projections are emitted
        # before this slot's attention phases so the scheduler can overlap
        phase1(0)
        for sl in range(NB):
            phase2(sl)
            if sl + 1 < NB:
                phase1(sl + 1)
            phase3(sl)

        # ---- phase 4: single BN all-reduce + apply ----
        cc_sb = statp.tile([128, 2 * G], f32, tag="ccsb")
        for g in range(G):
            nc.vector.tensor_reduce(cc_sb[:, g:g + 1], st1[g][:, 0:NB],
                                    axis=mybir.AxisListType.X, op=ALU.add)
            nc.vector.tensor_reduce(cc_sb[:, G + g:G + g + 1],
                                    st2[g][:, 0:NB],
                                    axis=mybir.AxisListType.X, op=ALU.add)
        cc_in = dramp.tile([128, 2 * G], f32, tag="ccin")
        red = statp.tile([128, 2 * G], f32, tag="red")
        nc.sync.dma_start(cc_in[:], cc_sb[:])
        if USE_CC:
            # AllGather + local reduce: shorter ncfw path than AllReduce.
            cc_out = dramp.tile([N_CORES, 128, 2 * G], f32, tag="ccout")
            nc.gpsimd.collective_compute(
                "AllGather", ALU.bypass,
                replica_groups=[list(range(N_CORES))],
                ins=[cc_in[:]], outs=[cc_out[:]],
            )
            red8 = statp.tile([128, 2 * G, N_CORES], f32, tag="red8")
            nc.sync.dma_start(
                red8[:],
                bass.AP(tensor=cc_out.tensor, offset=cc_out.offset,
                        ap=[[2 * G, 128], [1, 2 * G], [128 * 2 * G, N_CORES]]),
            )
            nc.vector.tensor_reduce(red[:], red8[:],
                                    axis=mybir.AxisListType.X, op=ALU.add)
        else:
            cc_out = dramp.tile([128, 2 * G], f32, tag="ccout")
            nc.sync.dma_start(cc_out[:], cc_in[:])
            nc.sync.dma_start(red[:], cc_out[:])

        mean = statp.tile([128, G], f32, tag="mean")
        nc.vector.tensor_scalar_mul(out=mean[:], in0=red[:, 0:G],
                                    scalar1=INV_BT)
        var = statp.tile([128, G], f32, tag="var")
        nc.vector.scalar_tensor_tensor(
            out=var[:], in0=mean[:], scalar=-1.0, in1=mean[:],
            op0=ALU.mult, op1=ALU.mult,
        )
        nc.vector.scalar_tensor_tensor(
            out=var[:], in0=red[:, G:2 * G], scalar=INV_BT,
            in1=var[:], op0=ALU.mult, op1=ALU.add,
        )
        sd = statp.tile([128, G], f32, tag="sd")
        nc.scalar.activation(sd[:], var[:], FT.Sqrt, bias=eps_t[:], scale=1.0)
        rs = statp.tile([128, G], f32, tag="rs")
        nc.vector.reciprocal(rs[:], sd[:])
        a_t = statp.tile([128, G], f32, tag="a_t")
        nc.vector.tensor_tensor(out=a_t[:], in0=gm_t, in1=rs[:], op=ALU.mult)
        bs_t = statp.tile([128, G], f32, tag="bs_t")
        nc.vector.scalar_tensor_tensor(
            out=bs_t[:], in0=mean[:], scalar=-1.0, in1=a_t[:],
            op0=ALU.mult, op1=ALU.mult,
        )
        nc.vector.tensor_tensor(out=bs_t[:], in0=bt_t, in1=bs_t[:], op=ALU.add)
        a_g = [a_t[:, g:g + 1] for g in range(G)]
        bs_g = [bs_t[:, g:g + 1] for g in range(G)]

        for sl in range(NB):
            Tj = slot_T[sl]
            ot = outp.tile([128, G, Tmax], f32, tag="ot", name=f"ot_{sl}")
            for g in range(G):
                nc.vector.tensor_scalar(
                    out=ot[:, g, :Tj], in0=xts[sl][g][:, :Tj],
                    scalar1=a_g[g], scalar2=bs_g[g],
                    op0=ALU.mult, op1=ALU.add, accum_out=None,
                )
            dst = bass.AP(
                tensor=out_d[sl].tensor, offset=out_d[sl].offset,
                ap=[[slot_T[sl], 128], [128 * slot_T[sl], G], [1, Tj]],
            )
            eng_o = nc.sync if sl % 2 == 0 else nc.scalar
            eng_o.dma_start(dst, ot[:, :, :Tj])

    nc.compile()
    return nc


_CACHE = {}


def _get_program(slot_T):
    key = tuple(slot_T)
    if key not in _CACHE:
        _CACHE[key] = _build(list(key))
    return _CACHE[key]


def kernel(seq, mask, Wq, bq, Wk, bk, Wv, bv, gamma, beta):
    import ml_dtypes
    bf = ml_dtypes.bfloat16
    seq = np.ascontiguousarray(np.asarray(seq, dtype=np.float32))
    mask_np = np.asarray(mask)
    counts = (mask_np != 0).sum(axis=1).astype(np.int64)
    order = np.argsort(-counts, kind="stable")

    # slot j on core c handles batch order[8*j + c]
    slot_T = []
    for j in range(NB):
        mx = int(counts[order[N_CORES * j:N_CORES * (j + 1)]].max())
        mx = (mx + 1) // 2 * 2  # fp32r matmuls need even free sizes
        slot_T.append(min(T, max(256, mx)))

    nc = _get_program(slot_T)

    wall = np.concatenate([
        np.asarray(Wq, np.float32).T, np.asarray(Wk, np.float32).T,
        np.asarray(Wv, np.float32).T,
    ], axis=1).astype(bf)
    ball = np.concatenate([
        np.asarray(bq, np.float32).reshape(-1),
        np.asarray(bk, np.float32).reshape(-1),
        np.asarray(gamma, np.float32).reshape(-1),
        np.asarray(beta, np.float32).reshape(-1),
    ])
    bvr = np.ascontiguousarray(np.asarray(bv, np.float32).reshape(1, C).astype(bf))
    bones = np.zeros((128, 128), bf)
    for r in range(4):
        bones[32 * r, 0:32] = 1.0
        bones[32 * r + 1, 64:96] = 1.0
    ones2 = np.tile(np.array([[1.0, 0.0]], np.float32), (1, H))
    onesr = np.ones((1, 128), bf)

    idx_map = {}
    in_maps = []
    for c in range(N_CORES):
        m = {
            "wall": wall, "ball": ball, "bvr": bvr,
            "bones": bones, "ones2": ones2, "onesr": onesr,
        }
        for j in range(NB):
            Tj = slot_T[j]
            sch = _cdiv(Tj, 128)
            b = int(order[N_CORES * j + c])
            idx = np.flatnonzero(mask_np[b] != 0)
            n = len(idx)
            idx_map[(c, j)] = (b, idx)
            sc = np.zeros((C, Tj), bf)
            sc[:, :n] = seq[b][:, idx].astype(bf)
            kb = np.full(sch * 128, KB_NEG, np.float32)
            kb[:n] = 0.0
            qm = np.zeros((1, Tj), np.float32)
            qm[:, :n] = 1.0
            m[f"seq{j}"] = sc
            m[f"kb{j}"] = kb
            m[f"qm{j}"] = qm
            if sch == 3 and Tj - 256 <= 32:
                # packed tail-chunk bias: partition 32a+k holds the key
                # bias of tail key 256+k (valid iff 256+k < n)
                kbt = np.full(128, KB_NEG, np.float32)
                m_t = Tj - 256
                for a in range(4):
                    for k in range(m_t):
                        if 256 + k < n:
                            kbt[32 * a + k] = 0.0
                m[f"kbt{j}"] = kbt
        in_maps.append(m)

    global _last_in_maps
    _last_in_maps = in_maps
    res = run_bass_kernel_spmd(nc, in_maps, core_ids=list(range(N_CORES)))

    out = np.zeros((B, C, T), np.float32)
    for c in range(N_CORES):
        for j in range(NB):
            b, idx = idx_map[(c, j)]
            n = len(idx)
            if n:
                out[b][:, idx] = res.results[c][f"out{j}"][:, :n]
    return out



# revision 2
# speedup vs baseline: 1.0929x; 1.0929x over previous
"""Trainium2 Bass kernel for nn_Attention_11579231830437.

Masked multi-head attention (ReLU'd QKV projections, additive key mask,
multiplicative query mask) followed by training-mode BatchNorm over (B, T).

Strategy: data-parallel over batch B across 8 NeuronCores (4 batches each).
The host compacts each batch's sequence to its valid (mask==1) positions —
both attention masks zero out the same positions, so attention is computed
only on the ~50% valid positions (sorted batch->slot assignment keeps the
per-slot padded length tight). BatchNorm mean/var sums are all-reduced
across cores on-chip (single collective at the end, with a warm-up
collective at kernel start to absorb ncfw first-trigger latency);
normalization is applied on-device before gathering.

On-device layout per batch slot (Tj = padded valid length of slot j):
  QT, KT   [C, Tj]   channel-major (per-partition bias + relu on ACT)
  V        [t, (h, dv | ones)]  row-major with a ones column per head, so the
           attention@V matmul also emits the softmax denominator row D.
  scores   ST[s, t] = KhT.T @ QhT per head (row-group tiled, K=32)
  P        exp(scale*ST + key_bias[s])  (key mask folded into the exp bias)
  out      O'[dv, t] accumulated over s-chunks; x = O' * (qmask/D) broadcast
           via a block-ones matmul; BN stats accumulate via fused reduces.
"""

import os
import sys
import types
from contextlib import ExitStack

import numpy as np

# Defensive: concourse.bass_utils imports antenv.axon_hooks when tracing is
# requested via env; provide a no-op holder if the image lacks it.
try:
    import antenv.axon_hooks  # noqa: F401
except Exception:
    try:
        import antenv

        _m = types.ModuleType("antenv.axon_hooks")
        _m._hook = None
        _m.set_axon_ntff_profile_hook = lambda h: setattr(_m, "_hook", h)
        _m.get_axon_ntff_profile_hook = lambda: getattr(_m, "_hook", None)
        sys.modules["antenv.axon_hooks"] = _m
        antenv.axon_hooks = _m
    except Exception:
        pass

import concourse.bass as bass
import concourse.bacc as bacc
import concourse.tile as tile
from concourse import mybir
from concourse.bass_utils import run_bass_kernel_spmd

f32 = mybir.dt.float32
bf16 = mybir.dt.bfloat16
FT = mybir.ActivationFunctionType
ALU = mybir.AluOpType

N_CORES = 8
B, C, T, H = 32, 256, 512, 8
D = C // H                      # 32 per-head dim
NB = B // N_CORES               # 4 batch slots per core
G = C // 128                    # 2 channel chunks of 128
EPS = 1e-5
SCALE = 1.0 / float(np.sqrt(D))
KB_NEG = -200.0                 # exp(-200 + few) == 0.0 in fp32
INV_BT = 1.0 / float(B * T)


def _cdiv(a, b):
    return (a + b - 1) // b


def _build(slot_T):
    """Build the SPMD program for per-slot padded lengths slot_T (NB ints)."""
    USE_CC = os.environ.get("K_NO_CC", "") == ""
    Tmax = max(slot_T)
    # slots whose 3rd key-chunk is tiny (<=32 keys): those keys are packed
    # into 32-row blocks (one per head quadrant) so the chunk costs one
    # exp of free-size Tj instead of four of 2*Tj.
    TAIL = {j for j in range(NB)
            if _cdiv(slot_T[j], 128) == 3 and slot_T[j] - 256 <= 32}
    nc = bacc.Bacc("TRN2", target_bir_lowering=False, debug=False,
                   num_devices=N_CORES)

    seq_d, kb_d, qm_d, out_d = [], [], [], []
    kbt_d = {}
    for j in range(NB):
        Tj = slot_T[j]
        sch = _cdiv(Tj, 128)
        seq_d.append(nc.dram_tensor(f"seq{j}", [C, Tj], bf16,
                                    kind="ExternalInput").ap())
        kb_d.append(nc.dram_tensor(f"kb{j}", [sch * 128], f32,
                                   kind="ExternalInput").ap())
        qm_d.append(nc.dram_tensor(f"qm{j}", [1, Tj], f32,
                                   kind="ExternalInput").ap())
        out_d.append(nc.dram_tensor(f"out{j}", [C, Tj], bf16,
                                    kind="ExternalOutput").ap())
        if j in TAIL:
            kbt_d[j] = nc.dram_tensor(f"kbt{j}", [128], f32,
                                      kind="ExternalInput").ap()

    wall_d = nc.dram_tensor("wall", [C, 3 * C], bf16, kind="ExternalInput").ap()
    ball_d = nc.dram_tensor("ball", [4 * C], f32, kind="ExternalInput").ap()
    bvr_d = nc.dram_tensor("bvr", [1, C], bf16, kind="ExternalInput").ap()
    bo_d = nc.dram_tensor("bones", [128, 128], bf16, kind="ExternalInput").ap()
    or_d = nc.dram_tensor("onesr", [1, 128], bf16, kind="ExternalInput").ap()
    on_d = nc.dram_tensor("ones2", [1, 2 * H], f32, kind="ExternalInput").ap()

    with tile.TileContext(nc) as tc, ExitStack() as ctx:
        const = ctx.enter_context(tc.tile_pool(name="const", bufs=1))
        seqp = ctx.enter_context(tc.tile_pool(name="seqp", bufs=3))
        qkp = ctx.enter_context(tc.tile_pool(name="qkp", bufs=3))
        vp = ctx.enter_context(tc.tile_pool(name="vp", bufs=3))
        pp = ctx.enter_context(tc.tile_pool(name="pp", bufs=3))
        xp = ctx.enter_context(tc.tile_pool(name="xp", bufs=NB))
        smallp = ctx.enter_context(tc.tile_pool(name="smallp", bufs=2))
        fbp = ctx.enter_context(tc.tile_pool(name="fbp", bufs=2))
        statp = ctx.enter_context(tc.tile_pool(name="statp", bufs=1))
        outp = ctx.enter_context(tc.tile_pool(name="outp", bufs=4))
        scrp = ctx.enter_context(tc.tile_pool(name="scrp", bufs=2))
        ps_proj = ctx.enter_context(tc.tile_pool(name="ps_proj", bufs=1, space="PSUM"))
        ps_sc = ctx.enter_context(tc.tile_pool(name="ps_sc", bufs=2, space="PSUM"))
        ps_av = ctx.enter_context(tc.tile_pool(name="ps_av", bufs=2, space="PSUM"))
        ps_fb = ctx.enter_context(tc.tile_pool(name="ps_fb", bufs=1, space="PSUM"))
        dramp = ctx.enter_context(tc.tile_pool(name="dramp", bufs=1, space="DRAM"))

        # ---- constants ----
        wpack = [const.tile([128, 3 * C], bf16, tag=f"wp{k}", name=f"wp{k}") for k in range(G)]
        nc.sync.dma_start(wpack[0][:], wall_d[0:128, :])
        nc.scalar.dma_start(wpack[1][:], wall_d[128:256, :])
        bpack = const.tile([128, 4, G], f32, tag="bpack")
        nc.sync.dma_start(
            bpack[:],
            bass.AP(tensor=ball_d.tensor, offset=ball_d.offset,
                    ap=[[1, 128], [128 * G, 4], [128, G]]),
        )
        bq_t = bpack[:, 0, :]
        bk_t = bpack[:, 1, :]
        gm_t = bpack[:, 2, :]
        bt_t = bpack[:, 3, :]
        wq_s = [wpack[k][:, 0:C] for k in range(G)]
        wk_s = [wpack[k][:, C:2 * C] for k in range(G)]
        wv_s = [wpack[k][:, 2 * C:3 * C] for k in range(G)]
        bones_b = const.tile([128, 128], bf16, tag="bones")
        nc.scalar.dma_start(bones_b[:], bo_d[:])
        bv_r = const.tile([1, C], bf16, tag="bvr32")
        nc.scalar.dma_start(bv_r[:], bvr_d[:])
        onesr = const.tile([1, 128], bf16, tag="onesr")
        nc.scalar.dma_start(onesr[:], or_d[:])
        eps_t = const.tile([128, 1], f32, tag="eps")
        nc.vector.memset(eps_t[:], EPS)
        zero_t = const.tile([128, Tmax], f32, tag="zerot")
        nc.gpsimd.memset(zero_t[:], 0.0)
        ones2_b = const.tile([128, H, 2], f32, tag="ones2b")
        nc.sync.dma_start(
            ones2_b[:],
            bass.AP(tensor=on_d.tensor, offset=on_d.offset,
                    ap=[[0, 128], [2, H], [1, 2]]),
        )

        st1 = [statp.tile([128, NB], f32, tag=f"st1_{g}", name=f"st1_{g}") for g in range(G)]
        st2 = [statp.tile([128, NB], f32, tag=f"st2_{g}", name=f"st2_{g}") for g in range(G)]

        # Warm-up collective: absorbs ncfw collectives-firmware warm-up off
        # the critical path (result unused).
        if USE_CC:
            warm_sb = const.tile([128, 2], f32, tag="warmsb")
            nc.vector.memset(warm_sb[:], 0.0)
            warm_in = dramp.tile([128, 2], f32, tag="warmin")
            warm_out = dramp.tile([128, 2], f32, tag="warmout")
            nc.sync.dma_start(warm_in[:], warm_sb[:])
            nc.gpsimd.collective_compute(
                "AllReduce", ALU.add,
                replica_groups=[list(range(N_CORES))],
                ins=[warm_in[:]], outs=[warm_out[:]],
            )

        xts = []  # [sl][g] -> XT tile
        S = [dict() for _ in range(NB)]

        def phase1(sl):
            Tj = slot_T[sl]
            sch = _cdiv(Tj, 128)
            msz = [min(128, Tj - 128 * i) for i in range(sch)]
            st = S[sl]
            st["sch"], st["msz"] = sch, msz

            s_in = [seqp.tile([128, Tmax], bf16, tag=f"sin{k}", name=f"sin{k}_{sl}") for k in range(G)]
            for k, eng in ((0, nc.sync), (1, nc.scalar)):
                eng.dma_start(s_in[k][:, :Tj],
                              seq_d[sl][128 * k:128 * (k + 1), :])
            kb_t = smallp.tile([128, 4], f32, tag="kb", name=f"kb_{sl}")
            nc.scalar.dma_start(
                kb_t[:, :sch],
                bass.AP(tensor=kb_d[sl].tensor, offset=kb_d[sl].offset,
                        ap=[[1, 128], [128, sch]]),
            )
            qm_b = smallp.tile([128, Tmax], f32, tag="qmb", name=f"qmb_{sl}")
            nc.sync.dma_start(
                qm_b[:, :Tj],
                bass.AP(tensor=qm_d[sl].tensor, offset=qm_d[sl].offset,
                        ap=[[0, 128], [1, Tj]]),
            )
            st["kb_t"], st["qm_b"] = kb_t, qm_b

            qt = [qkp.tile([128, Tmax], bf16, tag=f"qt{g}", name=f"qt{g}_{sl}") for g in range(G)]
            kt = [qkp.tile([128, Tmax], bf16, tag=f"kt{g}", name=f"kt{g}_{sl}") for g in range(G)]
            for g in range(G):
                for (w_s, b_t, dst) in ((wq_s, bq_t, qt), (wk_s, bk_t, kt)):
                    psq = ps_proj.tile([128, 512], f32, tag="psproj",
                                       name=f"psq_{sl}_{g}")
                    for k in range(G):
                        nc.tensor.matmul(
                            psq[:, :Tj],
                            w_s[k][:, 128 * g:128 * (g + 1)],
                            s_in[k][:, :Tj],
                            start=(k == 0), stop=(k == G - 1),
                        )
                    # relu(psq + bias) on DVE: (in0 + bias) max 0 — keeps the
                    # busier ACT engine free for exp.
                    nc.vector.scalar_tensor_tensor(
                        out=dst[g][:, :Tj], in0=psq[:, :Tj],
                        scalar=b_t[:, g:g + 1], in1=zero_t[:, :Tj],
                        op0=ALU.add, op1=ALU.max,
                    )
            st["qt"], st["kt"] = qt, kt

            v_t = [vp.tile([128, H, D + 2], bf16, tag=f"vt{i}", name=f"vt{i}_{sl}") for i in range(sch)]
            for i in range(sch):
                m = msz[i]
                psv = ps_proj.tile([128, 512], f32, tag="psproj",
                                   name=f"psv_{sl}_{i}")
                for k in range(G):
                    nc.tensor.matmul(
                        psv[:m, :C],
                        s_in[k][:, 128 * i:128 * i + m],
                        wv_s[k],
                        start=(k == 0), stop=False,
                    )
                nc.tensor.matmul(
                    psv[:m, :C], onesr[:, :m], bv_r[:],
                    start=False, stop=True,
                )
                nc.vector.tensor_scalar_max(
                    out=v_t[i][:m, :, 0:D],
                    in0=psv[:m, :C].rearrange("p (h d) -> p h d", h=H),
                    scalar1=0.0,
                )
                nc.vector.tensor_copy(v_t[i][:m, :, D:D + 2], ones2_b[:m, :, :])
            st["v_t"] = v_t
            if sl in TAIL:
                kbt_t = smallp.tile([128, 1], f32, tag="kbt", name=f"kbt_{sl}")
                nc.scalar.dma_start(
                    kbt_t[:],
                    bass.AP(tensor=kbt_d[sl].tensor, offset=kbt_d[sl].offset,
                            ap=[[1, 128], [128, 1]]),
                )
                # replicate the tail-chunk V rows to all four 32-row blocks
                vrep = vp.tile([128, H, D + 2], bf16, tag="vrep",
                               name=f"vrep_{sl}")
                vt = v_t[sch - 1]
                m_t = msz[sch - 1]
                for a in range(4):
                    (nc.sync if a % 2 == 0 else nc.scalar).dma_start(
                        vrep[32 * a:32 * a + m_t, :, :], vt[:m_t, :, :])
                st["kbt_t"], st["vrep"] = kbt_t, vrep

        def phase2(sl):
            Tj = slot_T[sl]
            st = S[sl]
            sch, msz = st["sch"], st["msz"]
            qt, kt, kb_t = st["qt"], st["kt"], st["kb_t"]
            has_tail = sl in TAIL
            ii = sch - 1 if has_tail else sch
            P = {}
            for i in range(ii):
                m = msz[i]
                for g in range(G):
                    for p_ in range(2):
                        ps2 = ps_sc.tile([128, 2, 512], f32, tag="pssc",
                                         name=f"ps2_{sl}_{i}_{g}_{p_}")
                        for jj in range(2):
                            j = 2 * p_ + jj
                            nc.tensor.matmul(
                                ps2[:m, jj, :Tj],
                                kt[g][32 * j:32 * (j + 1), 128 * i:128 * i + m],
                                qt[g][32 * j:32 * (j + 1), :Tj],
                                start=True, stop=True,
                                tile_position=(32 * j, 0),
                            )
                        pt = pp.tile([128, 2, Tmax], bf16, tag=f"p{i}{g}{p_}",
                                     name=f"p{i}{g}{p_}_{sl}")
                        nc.scalar.activation(
                            pt[:m, :, :Tj], ps2[:m, :, :Tj], FT.Exp,
                            bias=kb_t[:m, i:i + 1], scale=SCALE,
                        )
                        P[(i, g, 2 * p_)] = (pt, 0)
                        P[(i, g, 2 * p_ + 1)] = (pt, 1)
            st["P"] = P
            if has_tail:
                # packed tail round: head (g,j)'s <=32 tail keys land on
                # partitions 32j..; one exp of free-size Tj covers all four.
                # Unwritten partitions get bias -200 -> P=0.
                m_t = msz[sch - 1]
                kbt_t = st["kbt_t"]
                Pt = []
                for g in range(G):
                    pst = ps_sc.tile([128, 2, 512], f32, tag="pssc",
                                     name=f"pst_{sl}_{g}")
                    for j in range(4):
                        nc.tensor.matmul(
                            pst[32 * j:32 * j + m_t, 0, :Tj],
                            kt[g][32 * j:32 * (j + 1), 256:256 + m_t],
                            qt[g][32 * j:32 * (j + 1), :Tj],
                            start=True, stop=True,
                            tile_position=(32 * j, 32 * j),
                        )
                    ptt = pp.tile([128, Tmax], bf16, tag=f"ptt{g}",
                                  name=f"ptt{g}_{sl}")
                    nc.scalar.activation(
                        ptt[:, :Tj], pst[:, 0, :Tj], FT.Exp,
                        bias=kbt_t[:, 0:1], scale=SCALE,
                    )
                    Pt.append(ptt)
                st["Pt"] = Pt

        def phase3(sl):
            Tj = slot_T[sl]
            st = S[sl]
            sch, msz = st["sch"], st["msz"]
            v_t, P, qm_b = st["v_t"], st["P"], st["qm_b"]
            has_tail = sl in TAIL
            ii = sch - 1 if has_tail else sch
            m_t = msz[sch - 1] if has_tail else 0
            xt = [xp.tile([128, Tmax], f32, tag=f"xt{g}", name=f"xt{g}_{sl}") for g in range(G)]
            xts.append(xt)
            d_all = smallp.tile([128, Tmax], f32, tag="dall", name=f"dall_{sl}")
            nc.gpsimd.memset(d_all[:], 1.0)
            r_all = smallp.tile([128, Tmax], f32, tag="rall", name=f"rall_{sl}")
            f_all = smallp.tile([128, Tmax], bf16, tag="fall", name=f"fall_{sl}")
            a_sbs = {}
            for g in range(G):
                for p in range(2):
                    r = 2 * g + p
                    psA = ps_av.tile([128, 512], f32, tag="psav", name=f"psav{g}{p}_{sl}")
                    for pp_ in range(2):
                        h = 4 * g + 2 * p + pp_
                        base = 64 * pp_
                        j = 2 * p + pp_
                        for i in range(ii):
                            m = msz[i]
                            pt, jj = P[(i, g, 2 * p + pp_)]
                            nc.tensor.matmul(
                                psA[base:base + D + 2, :Tj],
                                v_t[i][:m, h, :],
                                pt[:m, jj, :Tj],
                                start=(i == 0),
                                stop=(i == ii - 1 and not has_tail),
                                tile_position=(0, base),
                            )
                        if has_tail:
                            nc.tensor.matmul(
                                psA[base:base + D + 2, :Tj],
                                st["vrep"][32 * j:32 * j + m_t, h, :],
                                st["Pt"][g][32 * j:32 * j + m_t, :Tj],
                                start=False, stop=True,
                                tile_position=(32 * j, base),
                            )
                    asb = fbp.tile([128, Tmax], f32, tag=f"asb{r}",
                                   name=f"asb{r}_{sl}")
                    a_sbs[r] = asb
                    if r % 2 == 0:
                        nc.vector.tensor_copy(asb[:, :Tj], psA[:, :Tj])
                    else:
                        nc.scalar.activation(asb[:, :Tj], psA[:, :Tj],
                                             FT.Copy)
                    pitch = asb.ap[0][0]
                    dsrc = bass.AP(
                        tensor=asb.tensor,
                        offset=asb.offset + D * pitch,
                        ap=[[64 * pitch, 2], [1, Tj]],
                    )
                    nc.sync.dma_start(d_all[32 * r:32 * r + 2, :Tj], dsrc)
            # D > 0 always holds: every query column (valid or padded) sees at
            # least one valid key with P >= exp(-|s|) > 0, so no eps guard.
            nc.vector.reciprocal(r_all[:, :Tj], d_all[:, :Tj])
            nc.vector.tensor_tensor(out=f_all[:, :Tj], in0=r_all[:, :Tj],
                                    in1=qm_b[:, :Tj], op=ALU.mult)
            for g in range(G):
                for p in range(2):
                    r = 2 * g + p
                    asb = a_sbs[r]
                    psF = ps_fb.tile([128, 512], f32, tag="psfb",
                                     name=f"psF_{r}_{sl}")
                    nc.tensor.matmul(psF[:, :Tj],
                                     bones_b[32 * r:32 * r + 2, :],
                                     f_all[32 * r:32 * r + 2, :Tj],
                                     start=True, stop=True,
                                     tile_position=(32 * r, 0))
                    for pp_ in range(2):
                        j = 2 * p + pp_
                        nc.vector.scalar_tensor_tensor(
                            out=xt[g][32 * j:32 * (j + 1), :Tj],
                            in0=asb[64 * pp_:64 * pp_ + D, :Tj],
                            scalar=1.0,
                            in1=psF[64 * pp_:64 * pp_ + D, :Tj],
                            op0=ALU.mult, op1=ALU.mult,
                            accum_out=st1[g][32 * j:32 * (j + 1), sl:sl + 1],
                        )
                scr = scrp.tile([128, Tmax], f32, tag="scr",
                                name=f"scr_{sl}_{g}")
                nc.scalar.activation(
                    scr[:, :Tj], xt[g][:, :Tj], FT.Square,
                    accum_out=st2[g][:, sl:sl + 1],
                )

        # software-pipelined emission: next slot's projections are emitted
        # before this slot's attention phases so the scheduler can overlap
        phase1(0)
        for sl in range(NB):
            phase2(sl)
            if sl + 1 < NB:
                phase1(sl + 1)
            phase3(sl)

        # ---- phase 4: single BN all-reduce + apply ----
        cc_sb = statp.tile([128, 2 * G], f32, tag="ccsb")
        for g in range(G):
            nc.vector.tensor_reduce(cc_sb[:, g:g + 1], st1[g][:, 0:NB],
                                    axis=mybir.AxisListType.X, op=ALU.add)
            nc.vector.tensor_reduce(cc_sb[:, G + g:G + g + 1],
                                    st2[g][:, 0:NB],
                                    axis=mybir.AxisListType.X, op=ALU.add)
        cc_in = dramp.tile([128, 2 * G], f32, tag="ccin")
        red = statp.tile([128, 2 * G], f32, tag="red")
        nc.sync.dma_start(cc_in[:], cc_sb[:])
        if USE_CC:
            # AllGather + local reduce: shorter ncfw path than AllReduce.
            cc_out = dramp.tile([N_CORES, 128, 2 * G], f32, tag="ccout")
            nc.gpsimd.collective_compute(
                "AllGather", ALU.bypass,
                replica_groups=[list(range(N_CORES))],
                ins=[cc_in[:]], outs=[cc_out[:]],
            )
            red8 = statp.tile([128, 2 * G, N_CORES], f32, tag="red8")
            nc.sync.dma_start(
                red8[:],
                bass.AP(tensor=cc_out.tensor, offset=cc_out.offset,
                        ap=[[2 * G, 128], [1, 2 * G], [128 * 2 * G, N_CORES]]),
            )
            nc.vector.tensor_reduce(red[:], red8[:],
                                    axis=mybir.AxisListType.X, op=ALU.add)
        else:
            cc_out = dramp.tile([128, 2 * G], f32, tag="ccout")
            nc.sync.dma_start(cc_out[:], cc_in[:])
            nc.sync.dma_start(red[:], cc_out[:])

        mean = statp.tile([128, G], f32, tag="mean")
        nc.vector.tensor_scalar_mul(out=mean[:], in0=red[:, 0:G],
                                    scalar1=INV_BT)
        var = statp.tile([128, G], f32, tag="var")
        nc.vector.scalar_tensor_tensor(
            out=var[:], in0=mean[:], scalar=-1.0, in1=mean[:],
            op0=ALU.mult, op1=ALU.mult,
        )
        nc.vector.scalar_tensor_tensor(
            out=var[:], in0=red[:, G:2 * G], scalar=INV_BT,
            in1=var[:], op0=ALU.mult, op1=ALU.add,
        )
        sd = statp.tile([128, G], f32, tag="sd")
        nc.scalar.activation(sd[:], var[:], FT.Sqrt, bias=eps_t[:], scale=1.0)
        rs = statp.tile([128, G], f32, tag="rs")
        nc.vector.reciprocal(rs[:], sd[:])
        a_t = statp.tile([128, G], f32, tag="a_t")
        nc.vector.tensor_tensor(out=a_t[:], in0=gm_t, in1=rs[:], op=ALU.mult)
        bs_t = statp.tile([128, G], f32, tag="bs_t")
        nc.vector.scalar_tensor_tensor(
            out=bs_t[:], in0=mean[:], scalar=-1.0, in1=a_t[:],
            op0=ALU.mult, op1=ALU.mult,
        )
        nc.vector.tensor_tensor(out=bs_t[:], in0=bt_t, in1=bs_t[:], op=ALU.add)
        a_g = [a_t[:, g:g + 1] for g in range(G)]
        bs_g = [bs_t[:, g:g + 1] for g in range(G)]

        for sl in range(NB):
            Tj = slot_T[sl]
            ot = outp.tile([128, G, Tmax], bf16, tag="ot", name=f"ot_{sl}")
            for g in range(G):
                nc.vector.tensor_scalar(
                    out=ot[:, g, :Tj], in0=xts[sl][g][:, :Tj],
                    scalar1=a_g[g], scalar2=bs_g[g],
                    op0=ALU.mult, op1=ALU.add, accum_out=None,
                )
            dst = bass.AP(
                tensor=out_d[sl].tensor, offset=out_d[sl].offset,
                ap=[[slot_T[sl], 128], [128 * slot_T[sl], G], [1, Tj]],
            )
            eng_o = (nc.sync, nc.scalar, nc.gpsimd)[sl % 3]
            eng_o.dma_start(dst, ot[:, :, :Tj])

    nc.compile()
    return nc


_CACHE = {}


def _get_program(slot_T):
    key = tuple(slot_T)
    if key not in _CACHE:
        _CACHE[key] = _build(list(key))
    return _CACHE[key]


def kernel(seq, mask, Wq, bq, Wk, bk, Wv, bv, gamma, beta):
    import ml_dtypes
    bf = ml_dtypes.bfloat16
    seq = np.ascontiguousarray(np.asarray(seq, dtype=np.float32))
    mask_np = np.asarray(mask)
    counts = (mask_np != 0).sum(axis=1).astype(np.int64)
    order = np.argsort(-counts, kind="stable")

    # slot j on core c handles batch order[8*j + c]
    slot_T = []
    for j in range(NB):
        mx = int(counts[order[N_CORES * j:N_CORES * (j + 1)]].max())
        mx = (mx + 1) // 2 * 2  # fp32r matmuls need even free sizes
        slot_T.append(min(T, max(256, mx)))

    nc = _get_program(slot_T)

    wall = np.concatenate([
        np.asarray(Wq, np.float32).T, np.asarray(Wk, np.float32).T,
        np.asarray(Wv, np.float32).T,
    ], axis=1).astype(bf)
    ball = np.concatenate([
        np.asarray(bq, np.float32).reshape(-1),
        np.asarray(bk, np.float32).reshape(-1),
        np.asarray(gamma, np.float32).reshape(-1),
        np.asarray(beta, np.float32).reshape(-1),
    ])
    bvr = np.ascontiguousarray(np.asarray(bv, np.float32).reshape(1, C).astype(bf))
    bones = np.zeros((128, 128), bf)
    for r in range(4):
        bones[32 * r, 0:32] = 1.0
        bones[32 * r + 1, 64:96] = 1.0
    ones2 = np.tile(np.array([[1.0, 0.0]], np.float32), (1, H))
    onesr = np.ones((1, 128), bf)

    idx_map = {}
    in_maps = []
    for c in range(N_CORES):
        m = {
            "wall": wall, "ball": ball, "bvr": bvr,
            "bones": bones, "ones2": ones2, "onesr": onesr,
        }
        for j in range(NB):
            Tj = slot_T[j]
            sch = _cdiv(Tj, 128)
            b = int(order[N_CORES * j + c])
            idx = np.flatnonzero(mask_np[b] != 0)
            n = len(idx)
            idx_map[(c, j)] = (b, idx)
            sc = np.zeros((C, Tj), bf)
            sc[:, :n] = seq[b][:, idx].astype(bf)
            kb = np.full(sch * 128, KB_NEG, np.float32)
            kb[:n] = 0.0
            qm = np.zeros((1, Tj), np.float32)
            qm[:, :n] = 1.0
            m[f"seq{j}"] = sc
            m[f"kb{j}"] = kb
            m[f"qm{j}"] = qm
            if sch == 3 and Tj - 256 <= 32:
                # packed tail-chunk bias: partition 32a+k holds the key
                # bias of tail key 256+k (valid iff 256+k < n)
                kbt = np.full(128, KB_NEG, np.float32)
                m_t = Tj - 256
                for a in range(4):
                    for k in range(m_t):
                        if 256 + k < n:
                            kbt[32 * a + k] = 0.0
                m[f"kbt{j}"] = kbt
        in_maps.append(m)

    global _last_in_maps
    _last_in_maps = in_maps
    res = run_bass_kernel_spmd(nc, in_maps, core_ids=list(range(N_CORES)))

    out = np.zeros((B, C, T), np.float32)
    for c in range(N_CORES):
        for j in range(NB):
            b, idx = idx_map[(c, j)]
            n = len(idx)
            if n:
                out[b][:, idx] = np.asarray(
                    res.results[c][f"out{j}"][:, :n], np.float32)
    return out



# revision 3
# speedup vs baseline: 1.1205x; 1.0253x over previous
"""Trainium2 Bass kernel for nn_Attention_11579231830437.

Masked multi-head attention (ReLU'd QKV projections, additive key mask,
multiplicative query mask) followed by training-mode BatchNorm over (B, T).

Strategy: data-parallel over batch B across 8 NeuronCores (4 batches each).
The host compacts each batch's sequence to its valid (mask==1) positions —
both attention masks zero out the same positions, so attention is computed
only on the ~50% valid positions (sorted batch->slot assignment keeps the
per-slot padded length tight). BatchNorm mean/var sums are all-reduced
across cores on-chip (single collective at the end, with a warm-up
collective at kernel start to absorb ncfw first-trigger latency);
normalization is applied on-device before gathering.

On-device layout per batch slot (Tj = padded valid length of slot j):
  QT, KT   [C, Tj]   channel-major (per-partition bias + relu on ACT)
  V        [t, (h, dv | ones)]  row-major with a ones column per head, so the
           attention@V matmul also emits the softmax denominator row D.
  scores   ST[s, t] = KhT.T @ QhT per head (row-group tiled, K=32)
  P        exp(scale*ST + key_bias[s])  (key mask folded into the exp bias)
  out      O'[dv, t] accumulated over s-chunks; x = O' * (qmask/D) broadcast
           via a block-ones matmul; BN stats accumulate via fused reduces.
"""

import os
import sys
import types
from contextlib import ExitStack

import numpy as np

# Defensive: concourse.bass_utils imports antenv.axon_hooks when tracing is
# requested via env; provide a no-op holder if the image lacks it.
try:
    import antenv.axon_hooks  # noqa: F401
except Exception:
    try:
        import antenv

        _m = types.ModuleType("antenv.axon_hooks")
        _m._hook = None
        _m.set_axon_ntff_profile_hook = lambda h: setattr(_m, "_hook", h)
        _m.get_axon_ntff_profile_hook = lambda: getattr(_m, "_hook", None)
        sys.modules["antenv.axon_hooks"] = _m
        antenv.axon_hooks = _m
    except Exception:
        pass

import concourse.bass as bass
import concourse.bacc as bacc
import concourse.tile as tile
from concourse import mybir
from concourse.bass_utils import run_bass_kernel_spmd

f32 = mybir.dt.float32
bf16 = mybir.dt.bfloat16
FT = mybir.ActivationFunctionType
ALU = mybir.AluOpType

N_CORES = 8
B, C, T, H = 32, 256, 512, 8
D = C // H                      # 32 per-head dim
NB = B // N_CORES               # 4 batch slots per core
G = C // 128                    # 2 channel chunks of 128
EPS = 1e-5
SCALE = 1.0 / float(np.sqrt(D))
KB_NEG = -200.0                 # exp(-200 + few) == 0.0 in fp32
INV_BT = 1.0 / float(B * T)


def _cdiv(a, b):
    return (a + b - 1) // b


def _build(slot_T):
    """Build the SPMD program for per-slot padded lengths slot_T (NB ints)."""
    USE_CC = os.environ.get("K_NO_CC", "") == ""
    Tmax = max(slot_T)
    # slots whose 3rd key-chunk is tiny (<=32 keys): those keys are packed
    # into 32-row blocks (one per head quadrant) so the chunk costs one
    # exp of free-size Tj instead of four of 2*Tj.
    TAIL = {j for j in range(NB)
            if _cdiv(slot_T[j], 128) == 3 and slot_T[j] - 256 <= 32}
    nc = bacc.Bacc("TRN2", target_bir_lowering=False, debug=False,
                   num_devices=N_CORES)

    seq_d, kb_d, qm_d, out_d = [], [], [], []
    kbt_d = {}
    for j in range(NB):
        Tj = slot_T[j]
        sch = _cdiv(Tj, 128)
        seq_d.append(nc.dram_tensor(f"seq{j}", [C, Tj], bf16,
                                    kind="ExternalInput").ap())
        kb_d.append(nc.dram_tensor(f"kb{j}", [sch * 128], f32,
                                   kind="ExternalInput").ap())
        qm_d.append(nc.dram_tensor(f"qm{j}", [1, Tj], f32,
                                   kind="ExternalInput").ap())
        out_d.append(nc.dram_tensor(f"out{j}", [C, Tj], bf16,
                                    kind="ExternalOutput").ap())
        if j in TAIL:
            kbt_d[j] = nc.dram_tensor(f"kbt{j}", [128], f32,
                                      kind="ExternalInput").ap()

    wall_d = nc.dram_tensor("wall", [C, 3 * C], bf16, kind="ExternalInput").ap()
    ball_d = nc.dram_tensor("ball", [4 * C], f32, kind="ExternalInput").ap()
    bvr_d = nc.dram_tensor("bvr", [1, C], bf16, kind="ExternalInput").ap()
    bo_d = nc.dram_tensor("bones", [128, 128], bf16, kind="ExternalInput").ap()
    or_d = nc.dram_tensor("onesr", [1, 128], bf16, kind="ExternalInput").ap()
    on_d = nc.dram_tensor("ones2", [1, 2 * H], f32, kind="ExternalInput").ap()

    with tile.TileContext(nc) as tc, ExitStack() as ctx:
        const = ctx.enter_context(tc.tile_pool(name="const", bufs=1))
        seqp = ctx.enter_context(tc.tile_pool(name="seqp", bufs=3))
        qkp = ctx.enter_context(tc.tile_pool(name="qkp", bufs=3))
        vp = ctx.enter_context(tc.tile_pool(name="vp", bufs=3))
        pp = ctx.enter_context(tc.tile_pool(name="pp", bufs=3))
        xp = ctx.enter_context(tc.tile_pool(name="xp", bufs=NB))
        smallp = ctx.enter_context(tc.tile_pool(name="smallp", bufs=2))
        fbp = ctx.enter_context(tc.tile_pool(name="fbp", bufs=2))
        statp = ctx.enter_context(tc.tile_pool(name="statp", bufs=1))
        outp = ctx.enter_context(tc.tile_pool(name="outp", bufs=4))
        scrp = ctx.enter_context(tc.tile_pool(name="scrp", bufs=2))
        ps_proj = ctx.enter_context(tc.tile_pool(name="ps_proj", bufs=1, space="PSUM"))
        ps_sc = ctx.enter_context(tc.tile_pool(name="ps_sc", bufs=2, space="PSUM"))
        ps_av = ctx.enter_context(tc.tile_pool(name="ps_av", bufs=2, space="PSUM"))
        ps_fb = ctx.enter_context(tc.tile_pool(name="ps_fb", bufs=1, space="PSUM"))
        dramp = ctx.enter_context(tc.tile_pool(name="dramp", bufs=1, space="DRAM"))

        # ---- constants ----
        wpack = [const.tile([128, 3 * C], bf16, tag=f"wp{k}", name=f"wp{k}") for k in range(G)]
        nc.sync.dma_start(wpack[0][:], wall_d[0:128, :])
        nc.scalar.dma_start(wpack[1][:], wall_d[128:256, :])
        bpack = const.tile([128, 4, G], f32, tag="bpack")
        nc.sync.dma_start(
            bpack[:],
            bass.AP(tensor=ball_d.tensor, offset=ball_d.offset,
                    ap=[[1, 128], [128 * G, 4], [128, G]]),
        )
        bq_t = bpack[:, 0, :]
        bk_t = bpack[:, 1, :]
        gm_t = bpack[:, 2, :]
        bt_t = bpack[:, 3, :]
        wq_s = [wpack[k][:, 0:C] for k in range(G)]
        wk_s = [wpack[k][:, C:2 * C] for k in range(G)]
        wv_s = [wpack[k][:, 2 * C:3 * C] for k in range(G)]
        bones_b = const.tile([128, 128], bf16, tag="bones")
        nc.scalar.dma_start(bones_b[:], bo_d[:])
        bv_r = const.tile([1, C], bf16, tag="bvr32")
        nc.scalar.dma_start(bv_r[:], bvr_d[:])
        onesr = const.tile([1, 128], bf16, tag="onesr")
        nc.scalar.dma_start(onesr[:], or_d[:])
        eps_t = const.tile([128, 1], f32, tag="eps")
        nc.vector.memset(eps_t[:], EPS)
        zero_t = const.tile([128, Tmax], f32, tag="zerot")
        nc.gpsimd.memset(zero_t[:], 0.0)
        ones2_b = const.tile([128, H, 2], f32, tag="ones2b")
        nc.sync.dma_start(
            ones2_b[:],
            bass.AP(tensor=on_d.tensor, offset=on_d.offset,
                    ap=[[0, 128], [2, H], [1, 2]]),
        )

        st1 = [statp.tile([128, NB], f32, tag=f"st1_{g}", name=f"st1_{g}") for g in range(G)]
        st2 = [statp.tile([128, NB], f32, tag=f"st2_{g}", name=f"st2_{g}") for g in range(G)]

        # Warm-up collective: absorbs ncfw collectives-firmware warm-up off
        # the critical path (result unused).
        if USE_CC:
            warm_sb = const.tile([128, 2], f32, tag="warmsb")
            nc.vector.memset(warm_sb[:], 0.0)
            warm_in = dramp.tile([128, 2], f32, tag="warmin")
            warm_out = dramp.tile([128, 2], f32, tag="warmout")
            nc.sync.dma_start(warm_in[:], warm_sb[:])
            nc.gpsimd.collective_compute(
                "AllReduce", ALU.add,
                replica_groups=[list(range(N_CORES))],
                ins=[warm_in[:]], outs=[warm_out[:]],
            )

        xts = []  # [sl][g] -> XT tile
        S = [dict() for _ in range(NB)]

        def phase1(sl):
            Tj = slot_T[sl]
            sch = _cdiv(Tj, 128)
            msz = [min(128, Tj - 128 * i) for i in range(sch)]
            st = S[sl]
            st["sch"], st["msz"] = sch, msz

            s_in = [seqp.tile([128, Tmax], bf16, tag=f"sin{k}", name=f"sin{k}_{sl}") for k in range(G)]
            for k, eng in ((0, nc.sync), (1, nc.scalar)):
                eng.dma_start(s_in[k][:, :Tj],
                              seq_d[sl][128 * k:128 * (k + 1), :])
            kb_t = smallp.tile([128, 4], f32, tag="kb", name=f"kb_{sl}")
            nc.scalar.dma_start(
                kb_t[:, :sch],
                bass.AP(tensor=kb_d[sl].tensor, offset=kb_d[sl].offset,
                        ap=[[1, 128], [128, sch]]),
            )
            qm_b = smallp.tile([128, Tmax], f32, tag="qmb", name=f"qmb_{sl}")
            nc.sync.dma_start(
                qm_b[:, :Tj],
                bass.AP(tensor=qm_d[sl].tensor, offset=qm_d[sl].offset,
                        ap=[[0, 128], [1, Tj]]),
            )
            st["kb_t"], st["qm_b"] = kb_t, qm_b

            qt = [qkp.tile([128, Tmax], bf16, tag=f"qt{g}", name=f"qt{g}_{sl}") for g in range(G)]
            kt = [qkp.tile([128, Tmax], bf16, tag=f"kt{g}", name=f"kt{g}_{sl}") for g in range(G)]
            for g in range(G):
                for (w_s, b_t, dst) in ((wq_s, bq_t, qt), (wk_s, bk_t, kt)):
                    psq = ps_proj.tile([128, 512], f32, tag="psproj",
                                       name=f"psq_{sl}_{g}")
                    for k in range(G):
                        nc.tensor.matmul(
                            psq[:, :Tj],
                            w_s[k][:, 128 * g:128 * (g + 1)],
                            s_in[k][:, :Tj],
                            start=(k == 0), stop=(k == G - 1),
                        )
                    # relu(psq + bias) on DVE: (in0 + bias) max 0 — keeps the
                    # busier ACT engine free for exp.
                    nc.vector.scalar_tensor_tensor(
                        out=dst[g][:, :Tj], in0=psq[:, :Tj],
                        scalar=b_t[:, g:g + 1], in1=zero_t[:, :Tj],
                        op0=ALU.add, op1=ALU.max,
                    )
            st["qt"], st["kt"] = qt, kt

            v_t = [vp.tile([128, H, D + 2], bf16, tag=f"vt{i}", name=f"vt{i}_{sl}") for i in range(sch)]
            for i in range(sch):
                m = msz[i]
                psv = ps_proj.tile([128, 512], f32, tag="psproj",
                                   name=f"psv_{sl}_{i}")
                for k in range(G):
                    nc.tensor.matmul(
                        psv[:m, :C],
                        s_in[k][:, 128 * i:128 * i + m],
                        wv_s[k],
                        start=(k == 0), stop=False,
                    )
                nc.tensor.matmul(
                    psv[:m, :C], onesr[:, :m], bv_r[:],
                    start=False, stop=True,
                )
                nc.vector.tensor_scalar_max(
                    out=v_t[i][:m, :, 0:D],
                    in0=psv[:m, :C].rearrange("p (h d) -> p h d", h=H),
                    scalar1=0.0,
                )
                nc.vector.tensor_copy(v_t[i][:m, :, D:D + 2], ones2_b[:m, :, :])
            st["v_t"] = v_t
            if sl in TAIL:
                kbt_t = smallp.tile([128, 1], f32, tag="kbt", name=f"kbt_{sl}")
                nc.scalar.dma_start(
                    kbt_t[:],
                    bass.AP(tensor=kbt_d[sl].tensor, offset=kbt_d[sl].offset,
                            ap=[[1, 128], [128, 1]]),
                )
                # replicate the tail-chunk V rows to all four 32-row blocks
                vrep = vp.tile([128, H, D + 2], bf16, tag="vrep",
                               name=f"vrep_{sl}")
                vt = v_t[sch - 1]
                m_t = msz[sch - 1]
                for a in range(4):
                    (nc.sync if a % 2 == 0 else nc.scalar).dma_start(
                        vrep[32 * a:32 * a + m_t, :, :], vt[:m_t, :, :])
                st["kbt_t"], st["vrep"] = kbt_t, vrep

        def phase2(sl):
            Tj = slot_T[sl]
            st = S[sl]
            sch, msz = st["sch"], st["msz"]
            qt, kt, kb_t = st["qt"], st["kt"], st["kb_t"]
            has_tail = sl in TAIL
            ii = sch - 1 if has_tail else sch
            P = {}
            for i in range(ii):
                m = msz[i]
                for g in range(G):
                    for p_ in range(2):
                        ps2 = ps_sc.tile([128, 2, 512], f32, tag="pssc",
                                         name=f"ps2_{sl}_{i}_{g}_{p_}")
                        for jj in range(2):
                            j = 2 * p_ + jj
                            nc.tensor.matmul(
                                ps2[:m, jj, :Tj],
                                kt[g][32 * j:32 * (j + 1), 128 * i:128 * i + m],
                                qt[g][32 * j:32 * (j + 1), :Tj],
                                start=True, stop=True,
                                tile_position=(32 * j, 0),
                            )
                        pt = pp.tile([128, 2, Tmax], bf16, tag=f"p{i}{g}{p_}",
                                     name=f"p{i}{g}{p_}_{sl}")
                        nc.scalar.activation(
                            pt[:m, :, :Tj], ps2[:m, :, :Tj], FT.Exp,
                            bias=kb_t[:m, i:i + 1], scale=SCALE,
                        )
                        P[(i, g, 2 * p_)] = (pt, 0)
                        P[(i, g, 2 * p_ + 1)] = (pt, 1)
            st["P"] = P
            if has_tail:
                # packed tail round: head (g,j)'s <=32 tail keys land on
                # partitions 32j..; one exp of free-size Tj covers all four.
                # Unwritten partitions get bias -200 -> P=0.
                m_t = msz[sch - 1]
                kbt_t = st["kbt_t"]
                Pt = []
                for g in range(G):
                    pst = ps_sc.tile([128, 2, 512], f32, tag="pssc",
                                     name=f"pst_{sl}_{g}")
                    for j in range(4):
                        nc.tensor.matmul(
                            pst[32 * j:32 * j + m_t, 0, :Tj],
                            kt[g][32 * j:32 * (j + 1), 256:256 + m_t],
                            qt[g][32 * j:32 * (j + 1), :Tj],
                            start=True, stop=True,
                            tile_position=(32 * j, 32 * j),
                        )
                    ptt = pp.tile([128, Tmax], bf16, tag=f"ptt{g}",
                                  name=f"ptt{g}_{sl}")
                    nc.scalar.activation(
                        ptt[:, :Tj], pst[:, 0, :Tj], FT.Exp,
                        bias=kbt_t[:, 0:1], scale=SCALE,
                    )
                    Pt.append(ptt)
                st["Pt"] = Pt

        def phase3(sl):
            Tj = slot_T[sl]
            st = S[sl]
            sch, msz = st["sch"], st["msz"]
            v_t, P, qm_b = st["v_t"], st["P"], st["qm_b"]
            has_tail = sl in TAIL
            ii = sch - 1 if has_tail else sch
            m_t = msz[sch - 1] if has_tail else 0
            xt = [xp.tile([128, Tmax], f32, tag=f"xt{g}", name=f"xt{g}_{sl}") for g in range(G)]
            xts.append(xt)
            d_all = smallp.tile([128, Tmax], f32, tag="dall", name=f"dall_{sl}")
            nc.gpsimd.memset(d_all[:], 1.0)
            r_all = smallp.tile([128, Tmax], f32, tag="rall", name=f"rall_{sl}")
            f_all = smallp.tile([128, Tmax], bf16, tag="fall", name=f"fall_{sl}")
            a_sbs = {}
            for g in range(G):
                for p in range(2):
                    r = 2 * g + p
                    psA = ps_av.tile([128, 512], f32, tag="psav", name=f"psav{g}{p}_{sl}")
                    for pp_ in range(2):
                        h = 4 * g + 2 * p + pp_
                        base = 64 * pp_
                        j = 2 * p + pp_
                        for i in range(ii):
                            m = msz[i]
                            pt, jj = P[(i, g, 2 * p + pp_)]
                            nc.tensor.matmul(
                                psA[base:base + D + 2, :Tj],
                                v_t[i][:m, h, :],
                                pt[:m, jj, :Tj],
                                start=(i == 0),
                                stop=(i == ii - 1 and not has_tail),
                                tile_position=(0, base),
                            )
                        if has_tail:
                            nc.tensor.matmul(
                                psA[base:base + D + 2, :Tj],
                                st["vrep"][32 * j:32 * j + m_t, h, :],
                                st["Pt"][g][32 * j:32 * j + m_t, :Tj],
                                start=False, stop=True,
                                tile_position=(32 * j, base),
                            )
                    asb = fbp.tile([128, Tmax], f32, tag=f"asb{r}",
                                   name=f"asb{r}_{sl}")
                    a_sbs[r] = asb
                    if r % 2 == 0:
                        nc.vector.tensor_copy(asb[:, :Tj], psA[:, :Tj])
                    else:
                        nc.scalar.activation(asb[:, :Tj], psA[:, :Tj],
                                             FT.Copy)
                    pitch = asb.ap[0][0]
                    dsrc = bass.AP(
                        tensor=asb.tensor,
                        offset=asb.offset + D * pitch,
                        ap=[[64 * pitch, 2], [1, Tj]],
                    )
                    nc.sync.dma_start(d_all[32 * r:32 * r + 2, :Tj], dsrc)
            # D > 0 always holds: every query column (valid or padded) sees at
            # least one valid key with P >= exp(-|s|) > 0, so no eps guard.
            nc.vector.reciprocal(r_all[:, :Tj], d_all[:, :Tj])
            nc.vector.tensor_tensor(out=f_all[:, :Tj], in0=r_all[:, :Tj],
                                    in1=qm_b[:, :Tj], op=ALU.mult)
            for g in range(G):
                for p in range(2):
                    r = 2 * g + p
                    asb = a_sbs[r]
                    psF = ps_fb.tile([128, 512], f32, tag="psfb",
                                     name=f"psF_{r}_{sl}")
                    nc.tensor.matmul(psF[:, :Tj],
                                     bones_b[32 * r:32 * r + 2, :],
                                     f_all[32 * r:32 * r + 2, :Tj],
                                     start=True, stop=True,
                                     tile_position=(32 * r, 0))
                    for pp_ in range(2):
                        j = 2 * p + pp_
                        nc.vector.scalar_tensor_tensor(
                            out=xt[g][32 * j:32 * (j + 1), :Tj],
                            in0=asb[64 * pp_:64 * pp_ + D, :Tj],
                            scalar=1.0,
                            in1=psF[64 * pp_:64 * pp_ + D, :Tj],
                            op0=ALU.mult, op1=ALU.mult,
                            accum_out=st1[g][32 * j:32 * (j + 1), sl:sl + 1],
                        )
                scr = scrp.tile([128, Tmax], f32, tag="scr",
                                name=f"scr_{sl}_{g}")
                nc.scalar.activation(
                    scr[:, :Tj], xt[g][:, :Tj], FT.Square,
                    accum_out=st2[g][:, sl:sl + 1],
                )

        # software-pipelined emission: next slot's projections are emitted
        # before this slot's attention phases so the scheduler can overlap
        phase1(0)
        for sl in range(NB):
            phase2(sl)
            if sl + 1 < NB:
                phase1(sl + 1)
            phase3(sl)

        # ---- phase 4: single BN all-reduce + apply ----
        cc_sb = statp.tile([128, 2 * G], f32, tag="ccsb")
        for g in range(G):
            nc.vector.tensor_reduce(cc_sb[:, g:g + 1], st1[g][:, 0:NB],
                                    axis=mybir.AxisListType.X, op=ALU.add)
            nc.vector.tensor_reduce(cc_sb[:, G + g:G + g + 1],
                                    st2[g][:, 0:NB],
                                    axis=mybir.AxisListType.X, op=ALU.add)
        cc_in = dramp.tile([128, 2 * G], f32, tag="ccin")
        red = statp.tile([128, 2 * G], f32, tag="red")
        nc.sync.dma_start(cc_in[:], cc_sb[:])
        if USE_CC:
            # AllGather + local reduce: shorter ncfw path than AllReduce.
            cc_out = dramp.tile([N_CORES, 128, 2 * G], f32, tag="ccout")
            nc.gpsimd.collective_compute(
                "AllGather", ALU.bypass,
                replica_groups=[list(range(N_CORES))],
                ins=[cc_in[:]], outs=[cc_out[:]],
            )
            red8 = statp.tile([128, 2 * G, N_CORES], f32, tag="red8")
            nc.sync.dma_start(
                red8[:],
                bass.AP(tensor=cc_out.tensor, offset=cc_out.offset,
                        ap=[[2 * G, 128], [1, 2 * G], [128 * 2 * G, N_CORES]]),
            )
            nc.vector.tensor_reduce(red[:], red8[:],
                                    axis=mybir.AxisListType.X, op=ALU.add)
        else:
            cc_out = dramp.tile([128, 2 * G], f32, tag="ccout")
            nc.sync.dma_start(cc_out[:], cc_in[:])
            nc.sync.dma_start(red[:], cc_out[:])

        mean = statp.tile([128, G], f32, tag="mean")
        nc.vector.tensor_scalar_mul(out=mean[:], in0=red[:, 0:G],
                                    scalar1=INV_BT)
        var = statp.tile([128, G], f32, tag="var")
        nc.vector.scalar_tensor_tensor(
            out=var[:], in0=mean[:], scalar=-1.0, in1=mean[:],
            op0=ALU.mult, op1=ALU.mult,
        )
        nc.vector.scalar_tensor_tensor(
            out=var[:], in0=red[:, G:2 * G], scalar=INV_BT,
            in1=var[:], op0=ALU.mult, op1=ALU.add,
        )
        sd = statp.tile([128, G], f32, tag="sd")
        nc.scalar.activation(sd[:], var[:], FT.Sqrt, bias=eps_t[:], scale=1.0)
        rs = statp.tile([128, G], f32, tag="rs")
        nc.vector.reciprocal(rs[:], sd[:])
        a_t = statp.tile([128, G], f32, tag="a_t")
        nc.vector.tensor_tensor(out=a_t[:], in0=gm_t, in1=rs[:], op=ALU.mult)
        bs_t = statp.tile([128, G], f32, tag="bs_t")
        nc.vector.scalar_tensor_tensor(
            out=bs_t[:], in0=mean[:], scalar=-1.0, in1=a_t[:],
            op0=ALU.mult, op1=ALU.mult,
        )
        nc.vector.tensor_tensor(out=bs_t[:], in0=bt_t, in1=bs_t[:], op=ALU.add)
        a_g = [a_t[:, g:g + 1] for g in range(G)]
        bs_g = [bs_t[:, g:g + 1] for g in range(G)]

        for sl in range(NB):
            Tj = slot_T[sl]
            ot = outp.tile([128, G, Tmax], bf16, tag="ot", name=f"ot_{sl}")
            for g in range(G):
                nc.vector.tensor_scalar(
                    out=ot[:, g, :Tj], in0=xts[sl][g][:, :Tj],
                    scalar1=a_g[g], scalar2=bs_g[g],
                    op0=ALU.mult, op1=ALU.add, accum_out=None,
                )
                # per-group DMA: starts as soon as this 128-row block is
                # applied instead of waiting for the whole slot
                dst = bass.AP(
                    tensor=out_d[sl].tensor,
                    offset=out_d[sl].offset + g * 128 * slot_T[sl],
                    ap=[[slot_T[sl], 128], [1, Tj]],
                )
                eng_o = (nc.sync, nc.scalar, nc.gpsimd)[(G * sl + g) % 3]
                eng_o.dma_start(dst, ot[:, g, :Tj])

    nc.compile()
    return nc


_CACHE = {}


def _get_program(slot_T):
    key = tuple(slot_T)
    if key not in _CACHE:
        _CACHE[key] = _build(list(key))
    return _CACHE[key]


def kernel(seq, mask, Wq, bq, Wk, bk, Wv, bv, gamma, beta):
    import ml_dtypes
    bf = ml_dtypes.bfloat16
    seq = np.ascontiguousarray(np.asarray(seq, dtype=np.float32))
    mask_np = np.asarray(mask)
    counts = (mask_np != 0).sum(axis=1).astype(np.int64)
    order = np.argsort(-counts, kind="stable")

    # slot j on core c handles batch order[8*j + c]
    slot_T = []
    for j in range(NB):
        mx = int(counts[order[N_CORES * j:N_CORES * (j + 1)]].max())
        mx = (mx + 1) // 2 * 2  # fp32r matmuls need even free sizes
        slot_T.append(min(T, max(256, mx)))

    nc = _get_program(slot_T)

    wall = np.concatenate([
        np.asarray(Wq, np.float32).T, np.asarray(Wk, np.float32).T,
        np.asarray(Wv, np.float32).T,
    ], axis=1).astype(bf)
    ball = np.concatenate([
        np.asarray(bq, np.float32).reshape(-1),
        np.asarray(bk, np.float32).reshape(-1),
        np.asarray(gamma, np.float32).reshape(-1),
        np.asarray(beta, np.float32).reshape(-1),
    ])
    bvr = np.ascontiguousarray(np.asarray(bv, np.float32).reshape(1, C).astype(bf))
    bones = np.zeros((128, 128), bf)
    for r in range(4):
        bones[32 * r, 0:32] = 1.0
        bones[32 * r + 1, 64:96] = 1.0
    ones2 = np.tile(np.array([[1.0, 0.0]], np.float32), (1, H))
    onesr = np.ones((1, 128), bf)

    idx_map = {}
    in_maps = []
    for c in range(N_CORES):
        m = {
            "wall": wall, "ball": ball, "bvr": bvr,
            "bones": bones, "ones2": ones2, "onesr": onesr,
        }
        for j in range(NB):
            Tj = slot_T[j]
            sch = _cdiv(Tj, 128)
            b = int(order[N_CORES * j + c])
            idx = np.flatnonzero(mask_np[b] != 0)
            n = len(idx)
            idx_map[(c, j)] = (b, idx)
            sc = np.zeros((C, Tj), bf)
            sc[:, :n] = seq[b][:, idx].astype(bf)
            kb = np.full(sch * 128, KB_NEG, np.float32)
            kb[:n] = 0.0
            qm = np.zeros((1, Tj), np.float32)
            qm[:, :n] = 1.0
            m[f"seq{j}"] = sc
            m[f"kb{j}"] = kb
            m[f"qm{j}"] = qm
            if sch == 3 and Tj - 256 <= 32:
                # packed tail-chunk bias: partition 32a+k holds the key
                # bias of tail key 256+k (valid iff 256+k < n)
                kbt = np.full(128, KB_NEG, np.float32)
                m_t = Tj - 256
                for a in range(4):
                    for k in range(m_t):
                        if 256 + k < n:
                            kbt[32 * a + k] = 0.0
                m[f"kbt{j}"] = kbt
        in_maps.append(m)

    global _last_in_maps
    _last_in_maps = in_maps
    res = run_bass_kernel_spmd(nc, in_maps, core_ids=list(range(N_CORES)))

    out = np.zeros((B, C, T), np.float32)
    for c in range(N_CORES):
        for j in range(NB):
            b, idx = idx_map[(c, j)]
            n = len(idx)
            if n:
                out[b][:, idx] = np.asarray(
                    res.results[c][f"out{j}"][:, :n], np.float32)
    return out



# revision 4
# speedup vs baseline: 1.1397x; 1.0171x over previous
"""Trainium2 Bass kernel for nn_Attention_11579231830437.

Masked multi-head attention (ReLU'd QKV projections, additive key mask,
multiplicative query mask) followed by training-mode BatchNorm over (B, T).

Strategy: data-parallel over batch B across 8 NeuronCores (4 batches each).
The host compacts each batch's sequence to its valid (mask==1) positions —
both attention masks zero out the same positions, so attention is computed
only on the ~50% valid positions (sorted batch->slot assignment keeps the
per-slot padded length tight). BatchNorm mean/var sums are all-reduced
across cores on-chip (single collective at the end, with a warm-up
collective at kernel start to absorb ncfw first-trigger latency);
normalization is applied on-device before gathering.

On-device layout per batch slot (Tj = padded valid length of slot j):
  QT, KT   [C, Tj]   channel-major (per-partition bias + relu on ACT)
  V        [t, (h, dv | ones)]  row-major with a ones column per head, so the
           attention@V matmul also emits the softmax denominator row D.
  scores   ST[s, t] = KhT.T @ QhT per head (row-group tiled, K=32)
  P        exp(scale*ST + key_bias[s])  (key mask folded into the exp bias)
  out      O'[dv, t] accumulated over s-chunks; x = O' * (qmask/D) broadcast
           via a block-ones matmul; BN stats accumulate via fused reduces.
"""

import os
import sys
import types
from contextlib import ExitStack

import numpy as np

# Defensive: concourse.bass_utils imports antenv.axon_hooks when tracing is
# requested via env; provide a no-op holder if the image lacks it.
try:
    import antenv.axon_hooks  # noqa: F401
except Exception:
    try:
        import antenv

        _m = types.ModuleType("antenv.axon_hooks")
        _m._hook = None
        _m.set_axon_ntff_profile_hook = lambda h: setattr(_m, "_hook", h)
        _m.get_axon_ntff_profile_hook = lambda: getattr(_m, "_hook", None)
        sys.modules["antenv.axon_hooks"] = _m
        antenv.axon_hooks = _m
    except Exception:
        pass

import concourse.bass as bass
import concourse.bacc as bacc
import concourse.tile as tile
from concourse import mybir
from concourse.bass_utils import run_bass_kernel_spmd

f32 = mybir.dt.float32
bf16 = mybir.dt.bfloat16
FT = mybir.ActivationFunctionType
ALU = mybir.AluOpType

N_CORES = 8
B, C, T, H = 32, 256, 512, 8
D = C // H                      # 32 per-head dim
NB = B // N_CORES               # 4 batch slots per core
G = C // 128                    # 2 channel chunks of 128
EPS = 1e-5
SCALE = 1.0 / float(np.sqrt(D))
KB_NEG = -200.0                 # exp(-200 + few) == 0.0 in fp32
INV_BT = 1.0 / float(B * T)


def _cdiv(a, b):
    return (a + b - 1) // b


def _build(slot_T):
    """Build the SPMD program for per-slot padded lengths slot_T (NB ints)."""
    USE_CC = os.environ.get("K_NO_CC", "") == ""
    Tmax = max(slot_T)
    # slots whose 3rd key-chunk is tiny (<=32 keys): those keys are packed
    # into 32-row blocks (one per head quadrant) so the chunk costs one
    # exp of free-size Tj instead of four of 2*Tj.
    TAIL = {j for j in range(NB)
            if _cdiv(slot_T[j], 128) == 3 and slot_T[j] - 256 <= 32}
    nc = bacc.Bacc("TRN2", target_bir_lowering=False, debug=False,
                   num_devices=N_CORES)

    seq_d, kb_d, qm_d, out_d = [], [], [], []
    kbt_d = {}
    for j in range(NB):
        Tj = slot_T[j]
        sch = _cdiv(Tj, 128)
        seq_d.append(nc.dram_tensor(f"seq{j}", [C, Tj], bf16,
                                    kind="ExternalInput").ap())
        kb_d.append(nc.dram_tensor(f"kb{j}", [sch * 128], f32,
                                   kind="ExternalInput").ap())
        qm_d.append(nc.dram_tensor(f"qm{j}", [1, Tj], f32,
                                   kind="ExternalInput").ap())
        out_d.append(nc.dram_tensor(f"out{j}", [C, Tj], bf16,
                                    kind="ExternalOutput").ap())
        if j in TAIL:
            kbt_d[j] = nc.dram_tensor(f"kbt{j}", [128], f32,
                                      kind="ExternalInput").ap()

    wall_d = nc.dram_tensor("wall", [C, 3 * C], bf16, kind="ExternalInput").ap()
    ball_d = nc.dram_tensor("ball", [4 * C], f32, kind="ExternalInput").ap()
    bvr_d = nc.dram_tensor("bvr", [1, C], bf16, kind="ExternalInput").ap()
    bo_d = nc.dram_tensor("bones", [128, 128], bf16, kind="ExternalInput").ap()
    or_d = nc.dram_tensor("onesr", [1, 128], bf16, kind="ExternalInput").ap()
    on_d = nc.dram_tensor("ones2", [1, 2 * H], f32, kind="ExternalInput").ap()

    with tile.TileContext(nc) as tc, ExitStack() as ctx:
        const = ctx.enter_context(tc.tile_pool(name="const", bufs=1))
        seqp = ctx.enter_context(tc.tile_pool(name="seqp", bufs=3))
        qkp = ctx.enter_context(tc.tile_pool(name="qkp", bufs=3))
        vp = ctx.enter_context(tc.tile_pool(name="vp", bufs=3))
        pp = ctx.enter_context(tc.tile_pool(name="pp", bufs=3))
        xp = ctx.enter_context(tc.tile_pool(name="xp", bufs=NB))
        smallp = ctx.enter_context(tc.tile_pool(name="smallp", bufs=2))
        fbp = ctx.enter_context(tc.tile_pool(name="fbp", bufs=2))
        statp = ctx.enter_context(tc.tile_pool(name="statp", bufs=1))
        outp = ctx.enter_context(tc.tile_pool(name="outp", bufs=4))
        scrp = ctx.enter_context(tc.tile_pool(name="scrp", bufs=2))
        ps_proj = ctx.enter_context(tc.tile_pool(name="ps_proj", bufs=1, space="PSUM"))
        ps_sc = ctx.enter_context(tc.tile_pool(name="ps_sc", bufs=2, space="PSUM"))
        ps_av = ctx.enter_context(tc.tile_pool(name="ps_av", bufs=2, space="PSUM"))
        ps_fb = ctx.enter_context(tc.tile_pool(name="ps_fb", bufs=1, space="PSUM"))
        dramp = ctx.enter_context(tc.tile_pool(name="dramp", bufs=1, space="DRAM"))

        # ---- constants ----
        wpack = [const.tile([128, 3 * C], bf16, tag=f"wp{k}", name=f"wp{k}") for k in range(G)]
        nc.sync.dma_start(wpack[0][:], wall_d[0:128, :])
        nc.scalar.dma_start(wpack[1][:], wall_d[128:256, :])
        bpack = const.tile([128, 4, G], f32, tag="bpack")
        nc.sync.dma_start(
            bpack[:],
            bass.AP(tensor=ball_d.tensor, offset=ball_d.offset,
                    ap=[[1, 128], [128 * G, 4], [128, G]]),
        )
        bq_t = bpack[:, 0, :]
        bk_t = bpack[:, 1, :]
        gm_t = bpack[:, 2, :]
        bt_t = bpack[:, 3, :]
        wq_s = [wpack[k][:, 0:C] for k in range(G)]
        wk_s = [wpack[k][:, C:2 * C] for k in range(G)]
        wv_s = [wpack[k][:, 2 * C:3 * C] for k in range(G)]
        bones_b = const.tile([128, 128], bf16, tag="bones")
        nc.scalar.dma_start(bones_b[:], bo_d[:])
        bv_r = const.tile([1, C], bf16, tag="bvr32")
        nc.scalar.dma_start(bv_r[:], bvr_d[:])
        onesr = const.tile([1, 128], bf16, tag="onesr")
        nc.scalar.dma_start(onesr[:], or_d[:])
        eps_t = const.tile([128, 1], f32, tag="eps")
        nc.vector.memset(eps_t[:], EPS)
        zero_t = const.tile([128, Tmax], f32, tag="zerot")
        nc.gpsimd.memset(zero_t[:], 0.0)
        ones2_b = const.tile([128, H, 2], f32, tag="ones2b")
        nc.sync.dma_start(
            ones2_b[:],
            bass.AP(tensor=on_d.tensor, offset=on_d.offset,
                    ap=[[0, 128], [2, H], [1, 2]]),
        )

        st1 = [statp.tile([128, NB], f32, tag=f"st1_{g}", name=f"st1_{g}") for g in range(G)]
        st2 = [statp.tile([128, NB], f32, tag=f"st2_{g}", name=f"st2_{g}") for g in range(G)]

        # Warm-up collective: absorbs ncfw collectives-firmware warm-up off
        # the critical path (result unused).
        if USE_CC:
            warm_sb = const.tile([128, 2], f32, tag="warmsb")
            nc.vector.memset(warm_sb[:], 0.0)
            warm_in = dramp.tile([128, 2], f32, tag="warmin")
            warm_out = dramp.tile([128, 2], f32, tag="warmout")
            nc.sync.dma_start(warm_in[:], warm_sb[:])
            nc.gpsimd.collective_compute(
                "AllReduce", ALU.add,
                replica_groups=[list(range(N_CORES))],
                ins=[warm_in[:]], outs=[warm_out[:]],
            )

        xts = []  # [sl][g] -> XT tile
        S = [dict() for _ in range(NB)]

        def phase1(sl):
            Tj = slot_T[sl]
            sch = _cdiv(Tj, 128)
            msz = [min(128, Tj - 128 * i) for i in range(sch)]
            st = S[sl]
            st["sch"], st["msz"] = sch, msz

            s_in = [seqp.tile([128, Tmax], bf16, tag=f"sin{k}", name=f"sin{k}_{sl}") for k in range(G)]
            for k, eng in ((0, nc.sync), (1, nc.scalar)):
                eng.dma_start(s_in[k][:, :Tj],
                              seq_d[sl][128 * k:128 * (k + 1), :])
            kb_t = smallp.tile([128, 4], f32, tag="kb", name=f"kb_{sl}")
            nc.scalar.dma_start(
                kb_t[:, :sch],
                bass.AP(tensor=kb_d[sl].tensor, offset=kb_d[sl].offset,
                        ap=[[1, 128], [128, sch]]),
            )
            qm_b = smallp.tile([128, Tmax], f32, tag="qmb", name=f"qmb_{sl}")
            nc.sync.dma_start(
                qm_b[:, :Tj],
                bass.AP(tensor=qm_d[sl].tensor, offset=qm_d[sl].offset,
                        ap=[[0, 128], [1, Tj]]),
            )
            st["kb_t"], st["qm_b"] = kb_t, qm_b

            qt = [qkp.tile([128, Tmax], bf16, tag=f"qt{g}", name=f"qt{g}_{sl}") for g in range(G)]
            kt = [qkp.tile([128, Tmax], bf16, tag=f"kt{g}", name=f"kt{g}_{sl}") for g in range(G)]
            for g in range(G):
                for (w_s, b_t, dst) in ((wq_s, bq_t, qt), (wk_s, bk_t, kt)):
                    psq = ps_proj.tile([128, 512], f32, tag="psproj",
                                       name=f"psq_{sl}_{g}")
                    for k in range(G):
                        nc.tensor.matmul(
                            psq[:, :Tj],
                            w_s[k][:, 128 * g:128 * (g + 1)],
                            s_in[k][:, :Tj],
                            start=(k == 0), stop=(k == G - 1),
                        )
                    # relu(psq + bias) on DVE: (in0 + bias) max 0 — keeps the
                    # busier ACT engine free for exp.
                    nc.vector.scalar_tensor_tensor(
                        out=dst[g][:, :Tj], in0=psq[:, :Tj],
                        scalar=b_t[:, g:g + 1], in1=zero_t[:, :Tj],
                        op0=ALU.add, op1=ALU.max,
                    )
            st["qt"], st["kt"] = qt, kt

            v_t = [vp.tile([128, H, D + 2], bf16, tag=f"vt{i}", name=f"vt{i}_{sl}") for i in range(sch)]
            for i in range(sch):
                m = msz[i]
                psv = ps_proj.tile([128, 512], f32, tag="psproj",
                                   name=f"psv_{sl}_{i}")
                for k in range(G):
                    nc.tensor.matmul(
                        psv[:m, :C],
                        s_in[k][:, 128 * i:128 * i + m],
                        wv_s[k],
                        start=(k == 0), stop=False,
                    )
                nc.tensor.matmul(
                    psv[:m, :C], onesr[:, :m], bv_r[:],
                    start=False, stop=True,
                )
                nc.vector.tensor_scalar_max(
                    out=v_t[i][:m, :, 0:D],
                    in0=psv[:m, :C].rearrange("p (h d) -> p h d", h=H),
                    scalar1=0.0,
                )
                nc.vector.tensor_copy(v_t[i][:m, :, D:D + 2], ones2_b[:m, :, :])
            st["v_t"] = v_t
            if sl in TAIL:
                kbt_t = smallp.tile([128, 1], f32, tag="kbt", name=f"kbt_{sl}")
                nc.scalar.dma_start(
                    kbt_t[:],
                    bass.AP(tensor=kbt_d[sl].tensor, offset=kbt_d[sl].offset,
                            ap=[[1, 128], [128, 1]]),
                )
                # replicate the tail-chunk V rows to all four 32-row blocks
                vrep = vp.tile([128, H, D + 2], bf16, tag="vrep",
                               name=f"vrep_{sl}")
                vt = v_t[sch - 1]
                m_t = msz[sch - 1]
                for a in range(4):
                    (nc.sync if a % 2 == 0 else nc.scalar).dma_start(
                        vrep[32 * a:32 * a + m_t, :, :], vt[:m_t, :, :])
                st["kbt_t"], st["vrep"] = kbt_t, vrep

        def phase2(sl):
            Tj = slot_T[sl]
            st = S[sl]
            sch, msz = st["sch"], st["msz"]
            qt, kt, kb_t = st["qt"], st["kt"], st["kb_t"]
            has_tail = sl in TAIL
            ii = sch - 1 if has_tail else sch
            P = {}
            for i in range(ii):
                m = msz[i]
                for g in range(G):
                    for p_ in range(2):
                        ps2 = ps_sc.tile([128, 2, 512], f32, tag="pssc",
                                         name=f"ps2_{sl}_{i}_{g}_{p_}")
                        for jj in range(2):
                            j = 2 * p_ + jj
                            nc.tensor.matmul(
                                ps2[:m, jj, :Tj],
                                kt[g][32 * j:32 * (j + 1), 128 * i:128 * i + m],
                                qt[g][32 * j:32 * (j + 1), :Tj],
                                start=True, stop=True,
                                tile_position=(32 * j, 0),
                            )
                        pt = pp.tile([128, 2, Tmax], bf16, tag=f"p{i}{g}{p_}",
                                     name=f"p{i}{g}{p_}_{sl}")
                        nc.scalar.activation(
                            pt[:m, :, :Tj], ps2[:m, :, :Tj], FT.Exp,
                            bias=kb_t[:m, i:i + 1], scale=SCALE,
                        )
                        P[(i, g, 2 * p_)] = (pt, 0)
                        P[(i, g, 2 * p_ + 1)] = (pt, 1)
            st["P"] = P
            if has_tail:
                # packed tail round: head (g,j)'s <=32 tail keys land on
                # partitions 32j..; one exp of free-size Tj covers all four.
                # Unwritten partitions get bias -200 -> P=0.
                m_t = msz[sch - 1]
                kbt_t = st["kbt_t"]
                Pt = []
                for g in range(G):
                    pst = ps_sc.tile([128, 2, 512], f32, tag="pssc",
                                     name=f"pst_{sl}_{g}")
                    for j in range(4):
                        nc.tensor.matmul(
                            pst[32 * j:32 * j + m_t, 0, :Tj],
                            kt[g][32 * j:32 * (j + 1), 256:256 + m_t],
                            qt[g][32 * j:32 * (j + 1), :Tj],
                            start=True, stop=True,
                            tile_position=(32 * j, 32 * j),
                        )
                    ptt = pp.tile([128, Tmax], bf16, tag=f"ptt{g}",
                                  name=f"ptt{g}_{sl}")
                    nc.scalar.activation(
                        ptt[:, :Tj], pst[:, 0, :Tj], FT.Exp,
                        bias=kbt_t[:, 0:1], scale=SCALE,
                    )
                    Pt.append(ptt)
                st["Pt"] = Pt

        def phase3(sl):
            Tj = slot_T[sl]
            st = S[sl]
            sch, msz = st["sch"], st["msz"]
            v_t, P, qm_b = st["v_t"], st["P"], st["qm_b"]
            has_tail = sl in TAIL
            ii = sch - 1 if has_tail else sch
            m_t = msz[sch - 1] if has_tail else 0
            xt = [xp.tile([128, Tmax], f32, tag=f"xt{g}", name=f"xt{g}_{sl}") for g in range(G)]
            xts.append(xt)
            d_all = smallp.tile([128, Tmax], f32, tag="dall", name=f"dall_{sl}")
            nc.gpsimd.memset(d_all[:], 1.0)
            r_all = smallp.tile([128, Tmax], f32, tag="rall", name=f"rall_{sl}")
            f_all = smallp.tile([128, Tmax], bf16, tag="fall", name=f"fall_{sl}")
            a_sbs = {}
            for g in range(G):
                for p in range(2):
                    r = 2 * g + p
                    psA = ps_av.tile([128, 512], f32, tag="psav", name=f"psav{g}{p}_{sl}")
                    for pp_ in range(2):
                        h = 4 * g + 2 * p + pp_
                        base = 64 * pp_
                        j = 2 * p + pp_
                        for i in range(ii):
                            m = msz[i]
                            pt, jj = P[(i, g, 2 * p + pp_)]
                            nc.tensor.matmul(
                                psA[base:base + D + 2, :Tj],
                                v_t[i][:m, h, :],
                                pt[:m, jj, :Tj],
                                start=(i == 0),
                                stop=(i == ii - 1 and not has_tail),
                                tile_position=(0, base),
                            )
                        if has_tail:
                            nc.tensor.matmul(
                                psA[base:base + D + 2, :Tj],
                                st["vrep"][32 * j:32 * j + m_t, h, :],
                                st["Pt"][g][32 * j:32 * j + m_t, :Tj],
                                start=False, stop=True,
                                tile_position=(32 * j, base),
                            )
                    asb = fbp.tile([128, Tmax], f32, tag=f"asb{r}",
                                   name=f"asb{r}_{sl}")
                    a_sbs[r] = asb
                    if r % 2 == 0:
                        nc.vector.tensor_copy(asb[:, :Tj], psA[:, :Tj])
                    else:
                        nc.scalar.activation(asb[:, :Tj], psA[:, :Tj],
                                             FT.Copy)
                    pitch = asb.ap[0][0]
                    dsrc = bass.AP(
                        tensor=asb.tensor,
                        offset=asb.offset + D * pitch,
                        ap=[[64 * pitch, 2], [1, Tj]],
                    )
                    nc.sync.dma_start(d_all[32 * r:32 * r + 2, :Tj], dsrc)
            # D > 0 always holds: every query column (valid or padded) sees at
            # least one valid key with P >= exp(-|s|) > 0, so no eps guard.
            nc.vector.reciprocal(r_all[:, :Tj], d_all[:, :Tj])
            nc.vector.tensor_tensor(out=f_all[:, :Tj], in0=r_all[:, :Tj],
                                    in1=qm_b[:, :Tj], op=ALU.mult)
            for g in range(G):
                for p in range(2):
                    r = 2 * g + p
                    asb = a_sbs[r]
                    psF = ps_fb.tile([128, 512], f32, tag="psfb",
                                     name=f"psF_{r}_{sl}")
                    nc.tensor.matmul(psF[:, :Tj],
                                     bones_b[32 * r:32 * r + 2, :],
                                     f_all[32 * r:32 * r + 2, :Tj],
                                     start=True, stop=True,
                                     tile_position=(32 * r, 0))
                    for pp_ in range(2):
                        j = 2 * p + pp_
                        nc.vector.scalar_tensor_tensor(
                            out=xt[g][32 * j:32 * (j + 1), :Tj],
                            in0=asb[64 * pp_:64 * pp_ + D, :Tj],
                            scalar=1.0,
                            in1=psF[64 * pp_:64 * pp_ + D, :Tj],
                            op0=ALU.mult, op1=ALU.mult,
                            accum_out=st1[g][32 * j:32 * (j + 1), sl:sl + 1],
                        )
                scr = scrp.tile([128, Tmax], f32, tag="scr",
                                name=f"scr_{sl}_{g}")
                nc.scalar.activation(
                    scr[:, :Tj], xt[g][:, :Tj], FT.Square,
                    accum_out=st2[g][:, sl:sl + 1],
                )

        # software-pipelined emission: next slot's projections are emitted
        # before this slot's attention phases so the scheduler can overlap
        phase1(0)
        for sl in range(NB):
            phase2(sl)
            if sl + 1 < NB:
                phase1(sl + 1)
            phase3(sl)

        # ---- phase 4: single BN all-reduce + apply ----
        cc_sb = statp.tile([128, 2 * G], f32, tag="ccsb")
        for g in range(G):
            nc.vector.tensor_reduce(cc_sb[:, g:g + 1], st1[g][:, 0:NB],
                                    axis=mybir.AxisListType.X, op=ALU.add)
            nc.vector.tensor_reduce(cc_sb[:, G + g:G + g + 1],
                                    st2[g][:, 0:NB],
                                    axis=mybir.AxisListType.X, op=ALU.add)
        cc_in = dramp.tile([128, 2 * G], f32, tag="ccin")
        red = statp.tile([128, 2 * G], f32, tag="red")
        nc.sync.dma_start(cc_in[:], cc_sb[:])
        if USE_CC:
            # AllGather + local reduce: shorter ncfw path than AllReduce.
            cc_out = dramp.tile([N_CORES, 128, 2 * G], f32, tag="ccout")
            nc.gpsimd.collective_compute(
                "AllGather", ALU.bypass,
                replica_groups=[list(range(N_CORES))],
                ins=[cc_in[:]], outs=[cc_out[:]],
            )
            red8 = statp.tile([128, 2 * G, N_CORES], f32, tag="red8")
            nc.sync.dma_start(
                red8[:],
                bass.AP(tensor=cc_out.tensor, offset=cc_out.offset,
                        ap=[[2 * G, 128], [1, 2 * G], [128 * 2 * G, N_CORES]]),
            )
            nc.vector.tensor_reduce(red[:], red8[:],
                                    axis=mybir.AxisListType.X, op=ALU.add)
        else:
            cc_out = dramp.tile([128, 2 * G], f32, tag="ccout")
            nc.sync.dma_start(cc_out[:], cc_in[:])
            nc.sync.dma_start(red[:], cc_out[:])

        mean = statp.tile([128, G], f32, tag="mean")
        nc.vector.tensor_scalar_mul(out=mean[:], in0=red[:, 0:G],
                                    scalar1=INV_BT)
        var = statp.tile([128, G], f32, tag="var")
        nc.vector.scalar_tensor_tensor(
            out=var[:], in0=mean[:], scalar=-1.0, in1=mean[:],
            op0=ALU.mult, op1=ALU.mult,
        )
        nc.vector.scalar_tensor_tensor(
            out=var[:], in0=red[:, G:2 * G], scalar=INV_BT,
            in1=var[:], op0=ALU.mult, op1=ALU.add,
        )
        sd = statp.tile([128, G], f32, tag="sd")
        nc.scalar.activation(sd[:], var[:], FT.Sqrt, bias=eps_t[:], scale=1.0)
        rs = statp.tile([128, G], f32, tag="rs")
        nc.vector.reciprocal(rs[:], sd[:])
        a_t = statp.tile([128, G], f32, tag="a_t")
        nc.vector.tensor_tensor(out=a_t[:], in0=gm_t, in1=rs[:], op=ALU.mult)
        bs_t = statp.tile([128, G], f32, tag="bs_t")
        nc.vector.scalar_tensor_tensor(
            out=bs_t[:], in0=mean[:], scalar=-1.0, in1=a_t[:],
            op0=ALU.mult, op1=ALU.mult,
        )
        nc.vector.tensor_tensor(out=bs_t[:], in0=bt_t, in1=bs_t[:], op=ALU.add)
        a_g = [a_t[:, g:g + 1] for g in range(G)]
        bs_g = [bs_t[:, g:g + 1] for g in range(G)]

        for sl in range(NB):
            Tj = slot_T[sl]
            ot = outp.tile([128, G, Tmax], bf16, tag="ot", name=f"ot_{sl}")
            for g in range(G):
                nc.vector.tensor_scalar(
                    out=ot[:, g, :Tj], in0=xts[sl][g][:, :Tj],
                    scalar1=a_g[g], scalar2=bs_g[g],
                    op0=ALU.mult, op1=ALU.add, accum_out=None,
                )
                # per-group DMA: starts as soon as this 128-row block is
                # applied instead of waiting for the whole slot
                dst = bass.AP(
                    tensor=out_d[sl].tensor,
                    offset=out_d[sl].offset + g * 128 * slot_T[sl],
                    ap=[[slot_T[sl], 128], [1, Tj]],
                )
                eng_o = (nc.sync, nc.scalar)[(G * sl + g) % 2]
                eng_o.dma_start(dst, ot[:, g, :Tj])

    nc.compile()
    return nc


_CACHE = {}


def _get_program(slot_T):
    key = tuple(slot_T)
    if key not in _CACHE:
        _CACHE[key] = _build(list(key))
    return _CACHE[key]


def kernel(seq, mask, Wq, bq, Wk, bk, Wv, bv, gamma, beta):
    import ml_dtypes
    bf = ml_dtypes.bfloat16
    seq = np.ascontiguousarray(np.asarray(seq, dtype=np.float32))
    mask_np = np.asarray(mask)
    counts = (mask_np != 0).sum(axis=1).astype(np.int64)
    order = np.argsort(-counts, kind="stable")

    # slot j on core c handles batch order[8*j + c]
    slot_T = []
    for j in range(NB):
        mx = int(counts[order[N_CORES * j:N_CORES * (j + 1)]].max())
        mx = (mx + 1) // 2 * 2  # fp32r matmuls need even free sizes
        slot_T.append(min(T, max(256, mx)))

    nc = _get_program(slot_T)

    wall = np.concatenate([
        np.asarray(Wq, np.float32).T, np.asarray(Wk, np.float32).T,
        np.asarray(Wv, np.float32).T,
    ], axis=1).astype(bf)
    ball = np.concatenate([
        np.asarray(bq, np.float32).reshape(-1),
        np.asarray(bk, np.float32).reshape(-1),
        np.asarray(gamma, np.float32).reshape(-1),
        np.asarray(beta, np.float32).reshape(-1),
    ])
    bvr = np.ascontiguousarray(np.asarray(bv, np.float32).reshape(1, C).astype(bf))
    bones = np.zeros((128, 128), bf)
    for r in range(4):
        bones[32 * r, 0:32] = 1.0
        bones[32 * r + 1, 64:96] = 1.0
    ones2 = np.tile(np.array([[1.0, 0.0]], np.float32), (1, H))
    onesr = np.ones((1, 128), bf)

    idx_map = {}
    in_maps = []
    for c in range(N_CORES):
        m = {
            "wall": wall, "ball": ball, "bvr": bvr,
            "bones": bones, "ones2": ones2, "onesr": onesr,
        }
        for j in range(NB):
            Tj = slot_T[j]
            sch = _cdiv(Tj, 128)
            b = int(order[N_CORES * j + c])
            idx = np.flatnonzero(mask_np[b] != 0)
            n = len(idx)
            idx_map[(c, j)] = (b, idx)
            sc = np.zeros((C, Tj), bf)
            sc[:, :n] = seq[b][:, idx].astype(bf)
            kb = np.full(sch * 128, KB_NEG, np.float32)
            kb[:n] = 0.0
            qm = np.zeros((1, Tj), np.float32)
            qm[:, :n] = 1.0
            m[f"seq{j}"] = sc
            m[f"kb{j}"] = kb
            m[f"qm{j}"] = qm
            if sch == 3 and Tj - 256 <= 32:
                # packed tail-chunk bias: partition 32a+k holds the key
                # bias of tail key 256+k (valid iff 256+k < n)
                kbt = np.full(128, KB_NEG, np.float32)
                m_t = Tj - 256
                for a in range(4):
                    for k in range(m_t):
                        if 256 + k < n:
                            kbt[32 * a + k] = 0.0
                m[f"kbt{j}"] = kbt
        in_maps.append(m)

    global _last_in_maps
    _last_in_maps = in_maps
    res = run_bass_kernel_spmd(nc, in_maps, core_ids=list(range(N_CORES)))

    out = np.zeros((B, C, T), np.float32)
    for c in range(N_CORES):
        for j in range(NB):
            b, idx = idx_map[(c, j)]
            n = len(idx)
            if n:
                out[b][:, idx] = np.asarray(
                    res.results[c][f"out{j}"][:, :n], np.float32)
    return out

